# revision 49
# baseline (speedup 1.0000x reference)
"""CycleNet Trainium2 kernel: 8-core data-parallel, host-routed pipeline.

v2: host pre-gathers all random-access rows between launches (device does only
sequential DMA), feature-major layouts (no per-block transposes), bf16 matmul
paths, batched DMAs, C merged into D-launch, F merged into last E-launch.
"""
import numpy as np
import ml_dtypes
import concourse.bass as bass
import concourse.tile as tile
from concourse import bacc, mybir
from concourse.bass_utils import run_bass_kernel_spmd
from concourse.masks import make_identity

F32 = mybir.dt.float32
BF16 = mybir.dt.bfloat16
NPBF = ml_dtypes.bfloat16
P = 128
RELU = mybir.ActivationFunctionType.Relu
COPY = mybir.ActivationFunctionType.Copy
EQ = mybir.AluOpType.is_equal

# problem constants
H = 128; N = 100000; E = 250000; N5 = 20000; N6 = 30000; G = 512; L = 3
AF = 9; AV = 64; BF = 3; BV = 8; BN_EPS = 1e-5
NC = 8
GPC = G // NC            # graphs per core
NB = 104                 # node blocks per core
NDP = NB * P             # padded local nodes = 13312
KE = 4                   # edge slot tiles per node block
NET = NB * KE            # edge slot tiles per core (416)
NP5 = 12500; NP5P = 12800; D5B = 20   # local c5 positions / padded / blocks
NP6 = 22500; NP6P = 23040; D6B = 30
K5 = 2                   # u5 slot tiles per node block
K6 = 3                   # u6 slot tiles per node block

_KER_CACHE = {}


def build_G():
    """Init embeddings via multi-hot matmuls: x0T, x5T, x6T (feature-major)."""
    nc = bacc.Bacc()
    atab = nc.dram_tensor("atab", [5, P, P], BF16, kind="ExternalInput")
    mh = nc.dram_tensor("mh", [5, P, NDP], BF16, kind="ExternalInput")
    ctab = nc.dram_tensor("ctab", [16, P], BF16, kind="ExternalInput")
    mh5 = nc.dram_tensor("mh5", [16, NP5P], BF16, kind="ExternalInput")
    mh6 = nc.dram_tensor("mh6", [16, NP6P], BF16, kind="ExternalInput")
    x0T = nc.dram_tensor("x0T", [P, NDP], BF16, kind="ExternalOutput")
    x5T = nc.dram_tensor("x5T", [P, NP5P], BF16, kind="ExternalOutput")
    x6T = nc.dram_tensor("x6T", [P, NP6P], BF16, kind="ExternalOutput")
    with tile.TileContext(nc) as tc:
        with (
            tc.tile_pool(name="cons", bufs=1) as cons,
            tc.tile_pool(name="sb", bufs=3) as sb,
            tc.tile_pool(name="ps", bufs=2, space="PSUM") as ps,
        ):
            at = [cons.tile([P, P], BF16, tag=f"at{t}", name=f"at{t}") for t in range(5)]
            for t in range(5):
                nc.sync.dma_start(out=at[t][:], in_=atab[t])
            ct = cons.tile([16, P], BF16)
            nc.sync.dma_start(out=ct[:], in_=ctab[:])
            # atoms: 4-block chunks
            for g in range(NB // 4):
                mt = sb.tile([P, 5, 4 * P], BF16, tag="mt")
                nc.scalar.dma_start(
                    out=mt[:], in_=mh[:, :, g * 4 * P:(g + 1) * 4 * P].rearrange("t p h -> p t h"))
                xp = ps.tile([P, 4 * P], F32, tag="xp")
                for t in range(5):
                    nc.tensor.matmul(out=xp[:], lhsT=at[t][:], rhs=mt[:, t, :],
                                     start=(t == 0), stop=(t == 4))
                xs = sb.tile([P, 4 * P], BF16, tag="xs")
                nc.vector.tensor_copy(out=xs[:], in_=xp[:])
                nc.sync.dma_start(out=x0T[:, g * 4 * P:(g + 1) * 4 * P], in_=xs[:])
            # cycles: per 640/768-block, halves
            for k, nblk, mhk, xko in ((5, D5B, mh5, x5T), (6, D6B, mh6, x6T)):
                hp = 64 * k
                for b in range(nblk):
                    m5 = sb.tile([16, 2 * hp], BF16, tag="m5")
                    nc.gpsimd.dma_start(out=m5[:], in_=mhk[:, b * 2 * hp:(b + 1) * 2 * hp])
                    xo = sb.tile([P, 2 * hp], BF16, tag="xo5")
                    for hh in range(2):
                        cp = ps.tile([P, hp], F32, tag="cp")
                        nc.tensor.matmul(out=cp[:], lhsT=ct[:], rhs=m5[:, hh * hp:(hh + 1) * hp],
                                         start=True, stop=True)
                        nc.vector.tensor_copy(out=xo[:, hh * hp:(hh + 1) * hp], in_=cp[:])
                    nc.sync.dma_start(out=xko[:, b * 2 * hp:(b + 1) * 2 * hp], in_=xo[:])
    nc.compile()
    return nc


def build_A():
    """Edge agg (pre-gathered) + GEMM1 (x-term folded) + BN1 stats."""
    nc = bacc.Bacc()
    sg = nc.dram_tensor("sg", [P, NET // 2, 2 * P], BF16, kind="ExternalInput")
    ohb = nc.dram_tensor("ohb", [P, NET // 2, 2 * P], BF16, kind="ExternalInput")
    xT = nc.dram_tensor("xT", [P, NDP], BF16, kind="ExternalInput")
    w1 = nc.dram_tensor("w1", [P, 2 * P], BF16, kind="ExternalInput")
    w1s = nc.dram_tensor("w1s", [P, 2 * P], BF16, kind="ExternalInput")
    t1T = nc.dram_tensor("t1T", [2 * P, NDP], BF16, kind="ExternalOutput")
    bstat = nc.dram_tensor("bstat", [2, P, 2], F32, kind="ExternalOutput")
    NG = NB // 4  # 26 groups of 4 blocks
    with tile.TileContext(nc) as tc:
        with (
            tc.tile_pool(name="cons", bufs=1) as cons,
            tc.tile_pool(name="xr", bufs=1) as xr,
            tc.tile_pool(name="sb", bufs=4) as sb,
            tc.tile_pool(name="st", bufs=1) as st,
            tc.tile_pool(name="ps", bufs=2, space="PSUM") as ps,
            tc.tile_pool(name="pt", bufs=2, space="PSUM") as pt,
        ):
            w1t = cons.tile([P, 2 * P], BF16)
            nc.sync.dma_start(out=w1t[:], in_=w1[:])
            w1st = cons.tile([P, 2 * P], BF16)
            nc.sync.dma_start(out=w1st[:], in_=w1s[:])
            xt = xr.tile([P, NDP], BF16)
            for j in range(13):
                nc.scalar.dma_start(out=xt[:, j * 1024:(j + 1) * 1024],
                                    in_=xT[:, j * 1024:(j + 1) * 1024])
            stat = [st.tile([P, NG, 6], F32, tag=f"sst{h}", name=f"sst{h}") for h in range(2)]
            for g in range(NG):
                agg4 = sb.tile([P, 4 * P], BF16, tag="agg4")
                for bb2 in range(2):
                    b2 = 2 * g + bb2  # 2-block chunk index
                    sgt = sb.tile([P, 4, 2, P], BF16, tag="sgt")
                    nc.gpsimd.dma_start(
                        out=sgt[:],
                        in_=sg[:, b2 * 4:(b2 + 1) * 4, :].rearrange("p t (i h) -> p t i h", i=2))
                    sl = sb.tile([P, 4, 2, P], BF16, tag="sl")
                    nc.scalar.activation(out=sl[:].rearrange("p t i h -> p (t i h)"),
                                         in_=sgt[:].rearrange("p t i h -> p (t i h)"), func=RELU)
                    oh = sb.tile([P, 4, 2, P], BF16, tag="oh")
                    nc.sync.dma_start(
                        out=oh[:],
                        in_=ohb[:, b2 * 4:(b2 + 1) * 4, :].rearrange("p t (i h) -> p t i h", i=2))
                    for bb in range(2):
                        aggT = ps.tile([P, P], F32, tag="aggT")
                        for t in range(KE):
                            tt = bb * 2 + t // 2
                            i = t % 2
                            nc.tensor.matmul(out=aggT[:], lhsT=sl[:, tt, i, :],
                                             rhs=oh[:, tt, i, :],
                                             start=(t == 0), stop=(t == KE - 1))
                        nc.scalar.activation(out=agg4[:, (2 * bb2 + bb) * P:(2 * bb2 + bb + 1) * P],
                                             in_=aggT[:], func=COPY)
                for half in range(2):
                    t1p = pt.tile([P, 4 * P], F32, tag=f"t1p{half}")
                    nc.tensor.matmul(out=t1p[:], lhsT=w1t[:, half * P:(half + 1) * P],
                                     rhs=agg4[:], start=True, stop=False)
                    nc.tensor.matmul(out=t1p[:], lhsT=w1st[:, half * P:(half + 1) * P],
                                     rhs=xt[:, g * 4 * P:(g + 1) * 4 * P], start=False, stop=True)
                    t1s = sb.tile([P, 4 * P], BF16, tag=f"t1s{half}")
                    nc.vector.tensor_copy(out=t1s[:], in_=t1p[:])
                    nc.vector.bn_stats(out=stat[half][:, g, :], in_=t1s[:])
                    nc.sync.dma_start(out=t1T[half * P:(half + 1) * P, g * 4 * P:(g + 1) * 4 * P],
                                      in_=t1s[:])
            for half in range(2):
                mv = sb.tile([P, 2], F32, tag="mv")
                nc.vector.bn_aggr(out=mv[:], in_=stat[half][:])
                nc.sync.dma_start(out=bstat[half], in_=mv[:])
    nc.compile()
    return nc


def build_B():
    """t2 = relu(t1*a1+b1); hT = w2-chain; BN2 stats."""
    nc = bacc.Bacc()
    t1T = nc.dram_tensor("t1T", [2 * P, NDP], BF16, kind="ExternalInput")
    ab1 = nc.dram_tensor("ab1", [2, 2, P, 1], F32, kind="ExternalInput")
    gw2 = nc.dram_tensor("gw2", [2 * P, P], BF16, kind="ExternalInput")
    hT = nc.dram_tensor("hT", [P, NDP], BF16, kind="ExternalOutput")
    bstat = nc.dram_tensor("bstat", [P, 2], F32, kind="ExternalOutput")
    NG = NB // 4
    with tile.TileContext(nc) as tc:
        with (
            tc.tile_pool(name="cons", bufs=1) as cons,
            tc.tile_pool(name="sb", bufs=4) as sb,
            tc.tile_pool(name="st", bufs=1) as st,
            tc.tile_pool(name="ps", bufs=2, space="PSUM") as ps,
        ):
            w2 = [cons.tile([P, P], BF16, tag=f"w2{h}", name=f"w2{h}") for h in range(2)]
            a1 = [cons.tile([P, 1], F32, tag=f"a{h}", name=f"a1_{h}") for h in range(2)]
            b1 = [cons.tile([P, 1], F32, tag=f"b{h}", name=f"b1_{h}") for h in range(2)]
            for h in range(2):
                nc.sync.dma_start(out=w2[h][:], in_=gw2[h * P:(h + 1) * P, :])
                nc.sync.dma_start(out=a1[h][:], in_=ab1[h, 0])
                nc.sync.dma_start(out=b1[h][:], in_=ab1[h, 1])
            stat = st.tile([P, NG, 6], F32)
            for g in range(NG):
                hp = ps.tile([P, 4 * P], F32, tag="hp")
                for half in range(2):
                    t1s = sb.tile([P, 4 * P], BF16, tag=f"t1s{half}")
                    (nc.scalar if half == 0 else nc.gpsimd).dma_start(
                        out=t1s[:], in_=t1T[half * P:(half + 1) * P,
                                            g * 4 * P:(g + 1) * 4 * P])
                    t2s = sb.tile([P, 4 * P], BF16, tag=f"t2s{half}")
                    nc.scalar.activation(out=t2s[:], in_=t1s[:], func=RELU,
                                         bias=b1[half][:], scale=a1[half][:])
                    nc.tensor.matmul(out=hp[:], lhsT=w2[half][:], rhs=t2s[:],
                                     start=(half == 0), stop=(half == 1))
                hs = sb.tile([P, 4 * P], BF16, tag="hs")
                nc.vector.tensor_copy(out=hs[:], in_=hp[:])
                nc.vector.bn_stats(out=stat[:, g, :], in_=hs[:])
                nc.sync.dma_start(out=hT[:, g * 4 * P:(g + 1) * 4 * P], in_=hs[:])
            mv = sb.tile([P, 2], F32, tag="mv")
            nc.vector.bn_aggr(out=mv[:], in_=stat[:])
            nc.sync.dma_start(out=bstat[:], in_=mv[:])
    nc.compile()
    return nc


def build_CD():
    """BN2-apply (xT out) + a2c mixes + cyclic path blocks for c5 and c6."""
    nc = bacc.Bacc()
    hT = nc.dram_tensor("hT", [P, NDP], BF16, kind="ExternalInput")
    ab2 = nc.dram_tensor("ab2", [2, P, 1], F32, kind="ExternalInput")
    z5g = nc.dram_tensor("z5g", [P, NP5P], BF16, kind="ExternalInput")
    z6g = nc.dram_tensor("z6g", [P, NP6P], BF16, kind="ExternalInput")
    x5T = nc.dram_tensor("x5T", [P, NP5P], BF16, kind="ExternalInput")
    x6T = nc.dram_tensor("x6T", [P, NP6P], BF16, kind="ExternalInput")
    aw5 = nc.dram_tensor("aw5", [P, P], BF16, kind="ExternalInput")
    ab5 = nc.dram_tensor("ab5", [P, 1], F32, kind="ExternalInput")
    aw6 = nc.dram_tensor("aw6", [P, P], BF16, kind="ExternalInput")
    ab6 = nc.dram_tensor("ab6", [P, 1], F32, kind="ExternalInput")
    pw5 = nc.dram_tensor("pw5", [3, P, P], BF16, kind="ExternalInput")
    pb5 = nc.dram_tensor("pb5", [P, 1], F32, kind="ExternalInput")
    pw6 = nc.dram_tensor("pw6", [3, P, P], BF16, kind="ExternalInput")
    pb6 = nc.dram_tensor("pb6", [P, 1], F32, kind="ExternalInput")
    xT = nc.dram_tensor("xT", [P, NDP], BF16, kind="ExternalOutput")
    x5To = nc.dram_tensor("x5To", [P, NP5P], BF16, kind="ExternalOutput")
    x6To = nc.dram_tensor("x6To", [P, NP6P], BF16, kind="ExternalOutput")
    with tile.TileContext(nc) as tc:
        with (
            tc.tile_pool(name="cons", bufs=1) as cons,
            tc.tile_pool(name="sb", bufs=4) as sb,
            tc.tile_pool(name="ps", bufs=3, space="PSUM") as ps,
        ):
            a2 = cons.tile([P, 1], F32)
            nc.sync.dma_start(out=a2[:], in_=ab2[0])
            b2 = cons.tile([P, 1], F32)
            nc.sync.dma_start(out=b2[:], in_=ab2[1])
            awt = {5: cons.tile([P, P], BF16, name="aw5t"), 6: cons.tile([P, P], BF16, name="aw6t")}
            abt = {5: cons.tile([P, 1], F32, name="ab5t"), 6: cons.tile([P, 1], F32, name="ab6t")}
            pwt = {5: [cons.tile([P, P], BF16, name=f"pw5{s}") for s in range(3)],
                   6: [cons.tile([P, P], BF16, name=f"pw6{s}") for s in range(3)]}
            pbt = {5: cons.tile([P, 1], F32, name="pb5t"), 6: cons.tile([P, 1], F32, name="pb6t")}
            for k, aws, abs_, pws, pbs in ((5, aw5, ab5, pw5, pb5), (6, aw6, ab6, pw6, pb6)):
                nc.sync.dma_start(out=awt[k][:], in_=aws[:])
                nc.sync.dma_start(out=abt[k][:], in_=abs_[:])
                for s in range(3):
                    nc.sync.dma_start(out=pwt[k][s][:], in_=pws[s])
                nc.sync.dma_start(out=pbt[k][:], in_=pbs[:])
            # C part: xT = relu(a2*hT + b2)
            for j in range(13):
                hs = sb.tile([P, 1024], BF16, tag="hs")
                nc.scalar.dma_start(out=hs[:], in_=hT[:, j * 1024:(j + 1) * 1024])
                xs = sb.tile([P, 1024], BF16, tag="xs")
                nc.scalar.activation(out=xs[:], in_=hs[:], func=RELU,
                                     bias=b2[:], scale=a2[:])
                nc.sync.dma_start(out=xT[:, j * 1024:(j + 1) * 1024], in_=xs[:])
            # D part per k: halo-layout conv, bf16 state
            for k, nblk, zg, xk, xko in ((5, D5B, z5g, x5T, x5To), (6, D6B, z6g, x6T, x6To)):
                BPOS = P * k
                HP = 64 * k
                for blk in range(nblk):
                    zt = sb.tile([P, BPOS], BF16, tag="zt")
                    nc.gpsimd.dma_start(out=zt[:], in_=zg[:, blk * BPOS:(blk + 1) * BPOS])
                    x5b = sb.tile([P, BPOS], BF16, tag="x5b")
                    nc.gpsimd.dma_start(out=x5b[:], in_=xk[:, blk * BPOS:(blk + 1) * BPOS])
                    xo = sb.tile([P, BPOS], BF16, tag="xo")
                    for hh in range(2):
                        zb = sb.tile([P, HP], BF16, tag="zb")
                        nc.scalar.activation(out=zb[:], in_=zt[:, hh * HP:(hh + 1) * HP],
                                             func=RELU, bias=b2[:], scale=a2[:])
                        rp = ps.tile([P, HP], F32, tag="rp")
                        nc.tensor.matmul(out=rp[:], lhsT=awt[k][:], rhs=zb[:],
                                         start=True, stop=True)
                        rs = sb.tile([P, HP], F32, tag="rs")
                        nc.vector.tensor_scalar(out=rs[:], in0=rp[:], scalar1=abt[k][:],
                                                scalar2=0.0, op0=mybir.AluOpType.add,
                                                op1=mybir.AluOpType.max)
                        xv3 = x5b[:, hh * HP:(hh + 1) * HP].rearrange("h (c j) -> h c j", j=k)
                        xch = sb.tile([P, 64, k + 2], BF16, tag="xch")
                        nc.vector.tensor_add(out=xch[:, :, 1:k + 1], in0=xv3,
                                             in1=rs[:].rearrange("h (c j) -> h c j", j=k))
                        nc.gpsimd.tensor_copy(out=xch[:, :, 0:1], in_=xch[:, :, k:k + 1])
                        nc.gpsimd.tensor_copy(out=xch[:, :, k + 1:k + 2], in_=xch[:, :, 1:2])
                        cvp = ps.tile([P, HP], F32, tag="cvp")
                        for s in range(3):
                            nc.tensor.matmul(out=cvp[:], lhsT=pwt[k][s][:],
                                             rhs=xch[:, :, s:s + k], start=(s == 0), stop=(s == 2))
                        cvr = sb.tile([P, HP], F32, tag="cvr")
                        nc.scalar.activation(out=cvr[:], in_=cvp[:], func=RELU, bias=pbt[k][:])
                        nc.vector.tensor_add(
                            out=xo[:, hh * HP:(hh + 1) * HP].rearrange("h (c j) -> h c j", j=k),
                            in0=xch[:, :, 1:k + 1], in1=cvr[:].rearrange("h (c j) -> h c j", j=k))
                    nc.sync.dma_start(out=xko[:, blk * BPOS:(blk + 1) * BPOS], in_=xo[:])
    nc.compile()
    return nc


def build_E(readout):
    """c2a: seg-mean (pre-gathered, pre-scaled) + linear + relu + residual.
    readout=True: fold the graph readout (F) in instead of storing xT'."""
    nc = bacc.Bacc()
    xT = nc.dram_tensor("xT", [P, NDP], BF16, kind="ExternalInput")
    u5g = nc.dram_tensor("u5g", [P, NB * K5 // 2, 2 * P], BF16, kind="ExternalInput")
    u6g = nc.dram_tensor("u6g", [P, NB * K6 // 2, 2 * P], BF16, kind="ExternalInput")
    drel5 = nc.dram_tensor("drel5", [P, NB * K5], F32, kind="ExternalInput")
    drel6 = nc.dram_tensor("drel6", [P, NB * K6], F32, kind="ExternalInput")
    iotaf = nc.dram_tensor("iotaf", [P, 8 * P], F32, kind="ExternalInput")
    w5 = nc.dram_tensor("w5", [P, P], BF16, kind="ExternalInput")
    b5 = nc.dram_tensor("b5", [P, 1], F32, kind="ExternalInput")
    w6 = nc.dram_tensor("w6", [P, P], BF16, kind="ExternalInput")
    b6 = nc.dram_tensor("b6", [P, 1], F32, kind="ExternalInput")
    if readout:
        grel = nc.dram_tensor("grel", [P, NB], F32, kind="ExternalInput")
        cig = nc.dram_tensor("cig", [P, GPC], F32, kind="ExternalInput")
        alw = nc.dram_tensor("alw", [P, P], F32, kind="ExternalInput")
        alb = nc.dram_tensor("alb", [P, 1], F32, kind="ExternalInput")
        linw = nc.dram_tensor("linw", [P, 1], F32, kind="ExternalInput")
        linb = nc.dram_tensor("linb", [1, 1], F32, kind="ExternalInput")
        y = nc.dram_tensor("y", [1, GPC], F32, kind="ExternalOutput")
    else:
        xTo = nc.dram_tensor("xTo", [P, NDP], BF16, kind="ExternalOutput")
    with tile.TileContext(nc) as tc:
        with (
            tc.tile_pool(name="cons", bufs=1) as cons,
            tc.tile_pool(name="sb", bufs=4) as sb,
            tc.tile_pool(name="psa", bufs=(1 if readout else 2), space="PSUM") as psa,
            tc.tile_pool(name="psb", bufs=(3 if readout else 2), space="PSUM") as psb,
            tc.tile_pool(name="psg", bufs=1, space="PSUM") as psg,
        ):
            iot8 = cons.tile([P, 8, P], F32)
            nc.sync.dma_start(out=iot8[:], in_=iotaf[:].rearrange("p (k h) -> p k h", k=8))
            dr5 = cons.tile([P, NB * K5], F32)
            nc.sync.dma_start(out=dr5[:], in_=drel5[:])
            dr6 = cons.tile([P, NB * K6], F32)
            nc.sync.dma_start(out=dr6[:], in_=drel6[:])
            wt = {5: cons.tile([P, P], BF16, name="w5t"), 6: cons.tile([P, P], BF16, name="w6t")}
            bt = {5: cons.tile([P, 1], F32, name="b5t"), 6: cons.tile([P, 1], F32, name="b6t")}
            nc.sync.dma_start(out=wt[5][:], in_=w5[:])
            nc.sync.dma_start(out=bt[5][:], in_=b5[:])
            nc.sync.dma_start(out=wt[6][:], in_=w6[:])
            nc.sync.dma_start(out=bt[6][:], in_=b6[:])
            if readout:
                ident = cons.tile([P, P], F32)
                make_identity(nc, ident[:])
                gr = cons.tile([P, NB], F32)
                nc.sync.dma_start(out=gr[:], in_=grel[:])
                cigt = cons.tile([P, GPC], F32)
                nc.sync.dma_start(out=cigt[:], in_=cig[:])
                alwt = cons.tile([P, P], F32)
                nc.sync.dma_start(out=alwt[:], in_=alw[:])
                albt = cons.tile([P, 1], F32)
                nc.sync.dma_start(out=albt[:], in_=alb[:])
                linwt = cons.tile([P, 1], F32)
                nc.sync.dma_start(out=linwt[:], in_=linw[:])
                linbt = cons.tile([1, 1], F32)
                nc.sync.dma_start(out=linbt[:], in_=linb[:])
                xgT = psg.tile([P, GPC], F32)
            for b in range(NB):
                if b % 4 == 0:
                    u5t = sb.tile([P, 4, 2, P], BF16, tag="u5t")
                    nc.scalar.dma_start(
                        out=u5t[:],
                        in_=u5g[:, b:b + 4, :].rearrange("p t (i h) -> p t i h", i=2))
                    xb = sb.tile([P, 4 * P], BF16, tag="xb")
                    nc.gpsimd.dma_start(out=xb[:], in_=xT[:, b * P:(b + 4) * P])
                    if not readout:
                        xno = sb.tile([P, 4 * P], BF16, tag="xno")
                if b % 2 == 0:
                    u6t = sb.tile([P, 3, 2, P], BF16, tag="u6t")
                    nc.gpsimd.dma_start(
                        out=u6t[:],
                        in_=u6g[:, b * 3 // 2:b * 3 // 2 + 3, :].rearrange(
                            "p t (i h) -> p t i h", i=2))
                rr = {}
                for k, K, ut, drk in ((5, K5, u5t, dr5), (6, K6, u6t, dr6)):
                    uT = psa.tile([P, P], F32, tag=f"uT{k}")
                    oh = sb.tile([P, K, P], BF16, tag=f"oh{k}")
                    nc.vector.tensor_tensor(
                        out=oh[:], in0=drk[:, b * K:(b + 1) * K].to_broadcast([P, K, P]),
                        in1=iot8[:, 0:K, :], op=EQ)
                    for t in range(K):
                        lt = (b % 4) * K + t if k == 5 else (b % 2) * K + t
                        nc.tensor.matmul(out=uT[:], lhsT=ut[:, lt // 2, lt % 2, :],
                                         rhs=oh[:, t, :],
                                         start=(t == 0), stop=(t == K - 1))
                    us = sb.tile([P, P], BF16, tag=f"us{k}")
                    if k == 5:
                        nc.vector.tensor_copy(out=us[:], in_=uT[:])
                    else:
                        nc.scalar.activation(out=us[:], in_=uT[:], func=COPY)
                    rp = psb.tile([P, P], F32, tag="rp")
                    nc.tensor.matmul(out=rp[:], lhsT=wt[k][:], rhs=us[:], start=True, stop=True)
                    rs = sb.tile([P, P], BF16, tag=f"rs{k}")
                    nc.scalar.activation(out=rs[:], in_=rp[:], func=RELU, bias=bt[k][:])
                    rr[k] = rs
                xn = sb.tile([P, P], BF16, tag="xn")
                nc.gpsimd.tensor_add(out=xn[:], in0=xb[:, (b % 4) * P:(b % 4 + 1) * P],
                                     in1=rr[5][:])
                if readout:
                    nc.vector.tensor_add(out=xn[:], in0=xn[:], in1=rr[6][:])
                    xf = sb.tile([P, P], F32, tag="xf")
                    nc.gpsimd.tensor_copy(out=xf[:], in_=xn[:])
                    tp = psb.tile([P, P], F32, tag="rp")
                    nc.tensor.transpose(out=tp[:], in_=xf[:], identity=ident[:])
                    xfT = sb.tile([P, P], F32, tag="xfT")
                    nc.vector.tensor_copy(out=xfT[:], in_=tp[:])
                    ohg = sb.tile([P, GPC], F32, tag="ohg")
                    nc.vector.tensor_tensor(out=ohg[:], in0=gr[:, b:b + 1].to_broadcast([P, GPC]),
                                            in1=iot8[:, 0, :GPC], op=EQ)
                    nc.tensor.matmul(out=xgT[:], lhsT=xfT[:], rhs=ohg[:],
                                     start=(b == 0), stop=(b == NB - 1))
                else:
                    nc.vector.tensor_add(out=xno[:, (b % 4) * P:(b % 4 + 1) * P],
                                         in0=xn[:], in1=rr[6][:])
                    if b % 4 == 3:
                        nc.sync.dma_start(out=xTo[:, (b - 3) * P:(b + 1) * P], in_=xno[:])
            if readout:
                xg = sb.tile([P, GPC], F32, tag="xg")
                nc.vector.tensor_mul(out=xg[:], in0=xgT[:], in1=cigt[:])
                ap = psg.tile([P, GPC], F32, tag="ap")
                nc.tensor.matmul(out=ap[:], lhsT=alwt[:], rhs=xg[:], start=True, stop=True)
                av = sb.tile([P, GPC], F32, tag="av")
                nc.scalar.activation(out=av[:], in_=ap[:], func=RELU, bias=albt[:])
                yp = psg.tile([1, GPC], F32, tag="yp")
                nc.tensor.matmul(out=yp[:], lhsT=linwt[:], rhs=av[:], start=True, stop=True)
                ys = sb.tile([1, GPC], F32, tag="ys")
                nc.vector.tensor_scalar_add(out=ys[:], in0=yp[:], scalar1=linbt[:])
                nc.sync.dma_start(out=y[:], in_=ys[:])
    nc.compile()
    return nc


def get_kernels():
    if "G" not in _KER_CACHE:
        _KER_CACHE.update(G=build_G(), A=build_A(), B=build_B(), CD=build_CD(),
                          E=build_E(False), E3=build_E(True))
    return _KER_CACHE


# ---------------------------------------------------------------- host glue

def slotmajor(vals, ntiles):
    """[ntiles*128, H] -> [128, ntiles//2, 2*H] bf16 (pair-interleaved)."""
    return np.ascontiguousarray(
        vals.reshape(ntiles // 2, 2, P, H).transpose(2, 0, 1, 3).reshape(
            P, ntiles // 2, 2 * H)).astype(NPBF)


class Prep:
    """Per-core layer-invariant index prep."""

    def __init__(self, x_atom, edge_index, edge_attr, batch, xc5, xc6, r5, r6):
        self.iotaf = np.tile(np.arange(P, dtype=np.float32)[None, :], (P, 8))
        core_of_node = (batch // GPC).astype(np.int64)
        self.node_lo = np.searchsorted(batch, np.arange(NC) * GPC)
        self.node_hi = np.searchsorted(batch, np.arange(NC) * GPC + GPC)
        self.nd = self.node_hi - self.node_lo
        assert self.nd.max() <= NDP
        src, dst = edge_index[0], edge_index[1]
        combo = (edge_attr[:, 0] * (BV * BV) + edge_attr[:, 1] * BV + edge_attr[:, 2])
        self.cores = []
        for c in range(NC):
            d = {}
            lo, hi, nd = self.node_lo[c], self.node_hi[c], self.nd[c]
            # ---- edge slots grouped by dst block
            em = np.where(core_of_node[dst] == c)[0]
            eblk = (dst[em] - lo) // P
            order = np.argsort(eblk, kind="stable")
            em = em[order]; eblk = eblk[order]
            cnt = np.bincount(eblk, minlength=NB)
            assert cnt.max() <= KE * P, f"edge block overflow {cnt.max()}"
            nslot = NET * P
            slot_src = np.zeros(nslot, dtype=np.int64)
            slot_ea = np.full(nslot, 512, dtype=np.int64)
            slot_dr = np.full(nslot, 255.0, dtype=np.float32)
            starts = np.concatenate([[0], np.cumsum(cnt)])
            for b in range(NB):
                sl = b * KE * P
                e = em[starts[b]:starts[b + 1]]
                slot_src[sl:sl + len(e)] = src[e]
                slot_ea[sl:sl + len(e)] = combo[e]
                slot_dr[sl:sl + len(e)] = (dst[e] - lo - b * P).astype(np.float32)
            d["slot_src"] = slot_src
            d["slot_ea"] = slot_ea
            dstrel = np.ascontiguousarray(slot_dr.reshape(NET, P).T)
            d["ohb"] = (dstrel[:, :, None]
                        == np.arange(P, dtype=np.float32)[None, None, :]).astype(
                            NPBF).reshape(P, NET // 2, 2 * P)
            # ---- z rows (a2c sources): global node ids per local cycle position
            for kk, npos, nposp, rows_all in ((5, NP5, NP5P, r5), (6, NP6, NP6P, r6)):
                rp = np.zeros(nposp, dtype=np.int64)
                rp[:npos] = rows_all[c * npos:(c + 1) * npos]
                d[f"z{kk}rows"] = rp
                d[f"z{kk}mask"] = npos
            # ---- u slots (c2a): positions targeting this core's nodes
            cnt5 = np.bincount(r5, minlength=N).astype(np.float32)
            cnt6 = np.bincount(r6, minlength=N).astype(np.float32)
            for kk, rows_all, K, cnt_node in ((5, r5, K5, cnt5), (6, r6, K6, cnt6)):
                pm = np.where(core_of_node[rows_all] == c)[0]
                tblk = (rows_all[pm] - lo) // P
                order = np.argsort(tblk, kind="stable")
                pm = pm[order]; tblk = tblk[order]
                cntb = np.bincount(tblk, minlength=NB)
                assert cntb.max() <= K * P, f"u{kk} block overflow {cntb.max()}"
                nslot = NB * K * P
                slot_pos = np.zeros(nslot, dtype=np.int64)
                slot_dr = np.full(nslot, 255.0, dtype=np.float32)
                slot_cs = np.zeros(nslot, dtype=np.float32)
                cinv = 1.0 / np.maximum(cnt_node, 1.0)
                st = np.concatenate([[0], np.cumsum(cntb)])
                for b in range(NB):
                    sl = b * K * P
                    pp = pm[st[b]:st[b + 1]]
                    slot_pos[sl:sl + len(pp)] = pp
                    slot_dr[sl:sl + len(pp)] = (rows_all[pp] - lo - b * P).astype(np.float32)
                    slot_cs[sl:sl + len(pp)] = cinv[rows_all[pp]]
                d[f"u{kk}pos"] = slot_pos
                d[f"u{kk}cs"] = slot_cs[:, None]
                d[f"drel{kk}"] = np.ascontiguousarray(slot_dr.reshape(NB * K, P).T)
            # ---- init multi-hots
            mh = np.zeros((640, NDP), dtype=np.float32)
            colr = np.arange(nd)
            for f in range(AF):
                mh[f * AV + x_atom[lo:hi, f], colr] = 1.0
            d["mh"] = np.ascontiguousarray(mh.reshape(5, P, NDP)).astype(NPBF)
            mh5 = np.zeros((16, NP5P), dtype=np.float32)
            mh5[xc5[c * NP5:(c + 1) * NP5], np.arange(NP5)] = 1.0
            d["mh5"] = mh5.astype(NPBF)
            mh6 = np.zeros((16, NP6P), dtype=np.float32)
            mh6[4 + xc6[c * NP6:(c + 1) * NP6], np.arange(NP6)] = 1.0
            d["mh6"] = mh6.astype(NPBF)
            # ---- readout
            grel = np.full((NB * P,), 255.0, dtype=np.float32)
            grel[:nd] = (batch[lo:hi] - c * GPC).astype(np.float32)
            d["grel"] = np.ascontiguousarray(grel.reshape(NB, P).T)
            gsz = np.bincount(batch, minlength=G).astype(np.float32)[c * GPC:(c + 1) * GPC]
            d["cig"] = np.tile(1.0 / np.maximum(gsz, 1.0)[None, :], (P, 1))
            self.cores.append(d)


def _run(nc, in_maps, trace=False):
    return run_bass_kernel_spmd(nc, in_maps, core_ids=list(range(NC)), trace=trace)


_EXEC_NS = []  # exec_time_ns per launch when tracing


def kernel(**inputs):
    inp = {k: np.asarray(v) for k, v in inputs.items()}
    x_atom = inp["x_atom"].astype(np.int64)
    edge_index = inp["edge_index"].astype(np.int64)
    edge_attr = inp["edge_attr"].astype(np.int64)
    batch = inp["batch"].astype(np.int64)
    xc5 = inp["xc5"].astype(np.int64); xc6 = inp["xc6"].astype(np.int64)
    r5 = inp["a2c5_row"].astype(np.int64); r6 = inp["a2c6_row"].astype(np.int64)
    f32 = lambda k: inp[k].astype(np.float32)
    atom_emb = f32("atom_emb"); bond_emb = f32("bond_emb")
    cyc5 = f32("cyc_emb5"); cyc6 = f32("cyc_emb6"); eps = f32("gine_eps")
    gw1 = f32("gw1"); gbn_g = f32("gbn_g"); gbn_b = f32("gbn_b")
    gw2 = f32("gw2"); bn_g = f32("bn_g"); bn_b = f32("bn_b")
    trace = bool(int(__import__("os").environ.get("CYC_TRACE", "0")))

    prep = Prep(x_atom, edge_index, edge_attr, batch, xc5, xc6, r5, r6)
    ks = get_kernels()
    _EXEC_NS.clear()

    def run(name, maps):
        res = _run(ks[name], maps, trace=trace)
        if trace and res.exec_time_ns is not None:
            _EXEC_NS.append((name, res.exec_time_ns))
        return res.results

    # ---- init embeddings
    atab = np.zeros((640, H), np.float32)
    atab[:AF * AV] = atom_emb.reshape(AF * AV, H)
    atab = np.ascontiguousarray(atab.reshape(5, P, H)).astype(NPBF)
    ctab = np.zeros((16, H), np.float32)
    ctab[0:4] = cyc5; ctab[4:8] = cyc6
    ctab = ctab.astype(NPBF)
    rG = run("G", [{"atab": atab, "ctab": ctab, "mh": d["mh"],
                    "mh5": d["mh5"], "mh6": d["mh6"]} for d in prep.cores])
    x_full = np.concatenate([
        np.asarray(rG[c]["x0T"]).astype(np.float32).T[:prep.nd[c]] for c in range(NC)])
    x5loc = [np.asarray(rG[c]["x5T"]) for c in range(NC)]
    x6loc = [np.asarray(rG[c]["x6T"]) for c in range(NC)]

    def xT_of(xf):
        """x_full [N,H] -> per-core zero-padded feature-major bf16 [P, NDP]."""
        outs = []
        for c in range(NC):
            m = np.zeros((NDP, H), np.float32)
            m[:prep.nd[c]] = xf[prep.node_lo[c]:prep.node_hi[c]]
            outs.append(np.ascontiguousarray(m.T).astype(NPBF))
        return outs

    for i in range(L):
        xTs = xT_of(x_full)
        be = bond_emb[i]
        combos = np.arange(BV ** 3)
        etab = (be[0][combos // (BV * BV)] + be[1][(combos // BV) % BV] + be[2][combos % BV])
        etab = np.concatenate([etab, np.zeros((1, H), np.float32)])
        w1 = gw1[i].astype(NPBF)
        w1s = (gw1[i] * (1.0 + eps[i])).astype(NPBF)
        # ---- A
        mapsA = []
        for c, d in enumerate(prep.cores):
            vals = x_full[d["slot_src"]] + etab[d["slot_ea"]]
            mapsA.append({"sg": slotmajor(vals, NET), "ohb": d["ohb"],
                          "xT": xTs[c], "w1": w1, "w1s": w1s})
        rA = run("A", mapsA)
        m = np.stack([np.concatenate([rA[c]["bstat"][0, :, 0], rA[c]["bstat"][1, :, 0]])
                      for c in range(NC)]).astype(np.float64)
        v = np.stack([np.concatenate([rA[c]["bstat"][0, :, 1], rA[c]["bstat"][1, :, 1]])
                      for c in range(NC)]).astype(np.float64)
        tot = m.sum(0) * NDP
        tot2 = (v + m ** 2).sum(0) * NDP
        m1 = tot / N
        v1 = tot2 / N - m1 ** 2
        a1 = (gbn_g[i] / np.sqrt(v1 + BN_EPS)).astype(np.float32)
        b1 = (gbn_b[i] - a1 * m1).astype(np.float32)
        ab1 = np.stack([np.stack([a1[h * P:(h + 1) * P, None], b1[h * P:(h + 1) * P, None]])
                        for h in range(2)])
        # ---- B
        rB = run("B", [{"t1T": rA[c]["t1T"], "ab1": ab1, "gw2": gw2[i].astype(NPBF)}
                       for c in range(NC)])
        m2 = np.stack([rB[c]["bstat"][:, 0] for c in range(NC)]).astype(np.float64)
        v2 = np.stack([rB[c]["bstat"][:, 1] for c in range(NC)]).astype(np.float64)
        hpad = (np.maximum(b1, 0.0).astype(np.float64) @ gw2[i].astype(np.float64))
        npad = NC * NDP - N
        tot = m2.sum(0) * NDP - npad * hpad
        tot2 = (v2 + m2 ** 2).sum(0) * NDP - npad * hpad ** 2
        m2g = tot / N
        v2g = tot2 / N - m2g ** 2
        a2 = (bn_g[i] / np.sqrt(v2g + BN_EPS)).astype(np.float32)
        b2 = (bn_b[i] - a2 * m2g).astype(np.float32)
        ab2 = np.stack([a2[:, None], b2[:, None]])
        # ---- CD
        h_full = np.concatenate([
            np.asarray(rB[c]["hT"]).astype(np.float32).T[:prep.nd[c]] for c in range(NC)])
        mapsCD = []
        for c, d in enumerate(prep.cores):
            z5 = np.ascontiguousarray(h_full[d["z5rows"]].T).astype(NPBF)
            z6 = np.ascontiguousarray(h_full[d["z6rows"]].T).astype(NPBF)
            mapsCD.append({"hT": rB[c]["hT"], "ab2": ab2, "z5g": z5, "z6g": z6,
                           "x5T": x5loc[c], "x6T": x6loc[c],
                           "aw5": f32("a2c5_w")[i].astype(NPBF),
                           "ab5": f32("a2c5_b")[i][:, None],
                           "aw6": f32("a2c6_w")[i].astype(NPBF),
                           "ab6": f32("a2c6_b")[i][:, None],
                           "pw5": f32("p5_w")[i].astype(NPBF),
                           "pb5": f32("p5_b")[i][:, None],
                           "pw6": f32("p6_w")[i].astype(NPBF),
                           "pb6": f32("p6_b")[i][:, None]})
        rCD = run("CD", mapsCD)
        for c in range(NC):
            x5loc[c] = np.asarray(rCD[c]["x5To"])
            x6loc[c] = np.asarray(rCD[c]["x6To"])
        x5_full = np.concatenate(
            [x5loc[c].astype(np.float32).T[:NP5] for c in range(NC)])
        x6_full = np.concatenate(
            [x6loc[c].astype(np.float32).T[:NP6] for c in range(NC)])
        # ---- E / E3
        last = (i == L - 1)
        mapsE = []
        for c, d in enumerate(prep.cores):
            u5 = x5_full[d["u5pos"]] * d["u5cs"]
            u6 = x6_full[d["u6pos"]] * d["u6cs"]
            me = {"xT": rCD[c]["xT"], "u5g": slotmajor(u5, NB * K5),
                  "u6g": slotmajor(u6, NB * K6),
                  "drel5": d["drel5"], "drel6": d["drel6"], "iotaf": prep.iotaf,
                  "w5": f32("c2a5_w")[i].astype(NPBF), "b5": f32("c2a5_b")[i][:, None],
                  "w6": f32("c2a6_w")[i].astype(NPBF), "b6": f32("c2a6_b")[i][:, None]}
            if last:
                me.update({"grel": d["grel"], "cig": d["cig"],
                           "alw": f32("atom_lin_w"), "alb": f32("atom_lin_b")[:, None],
                           "linw": f32("lin_w"), "linb": f32("lin_b")[None, :]})
            mapsE.append(me)
        rE = run("E3" if last else "E", mapsE)
        if not last:
            x_full = np.concatenate([
                np.asarray(rE[c]["xTo"]).astype(np.float32).T[:prep.nd[c]]
                for c in range(NC)])
    y = np.concatenate([rE[c]["y"][0] for c in range(NC)])[:, None]
    return y.astype(np.float32)


# revision 51
# speedup vs baseline: 1.0703x; 1.0703x over previous
"""CycleNet Trainium2 kernel: 8-core data-parallel, host-routed pipeline.

v2: host pre-gathers all random-access rows between launches (device does only
sequential DMA), feature-major layouts (no per-block transposes), bf16 matmul
paths, batched DMAs, C merged into D-launch, F merged into last E-launch.
"""
import numpy as np
import ml_dtypes
import concourse.bass as bass
import concourse.tile as tile
from concourse import bacc, mybir
from concourse.bass_utils import run_bass_kernel_spmd
from concourse.masks import make_identity

F32 = mybir.dt.float32
BF16 = mybir.dt.bfloat16
NPBF = ml_dtypes.bfloat16
P = 128
RELU = mybir.ActivationFunctionType.Relu
COPY = mybir.ActivationFunctionType.Copy
EQ = mybir.AluOpType.is_equal

# problem constants
H = 128; N = 100000; E = 250000; N5 = 20000; N6 = 30000; G = 512; L = 3
AF = 9; AV = 64; BF = 3; BV = 8; BN_EPS = 1e-5
NC = 8
GPC = G // NC            # graphs per core
NB = 104                 # node blocks per core
NDP = NB * P             # padded local nodes = 13312
KE = 4                   # edge slot tiles per node block
NET = NB * KE            # edge slot tiles per core (416)
NP5 = 12500; NP5P = 12800; D5B = 20   # local c5 positions / padded / blocks
NP6 = 22500; NP6P = 23040; D6B = 30
K5 = 2                   # u5 slot tiles per node block
K6 = 3                   # u6 slot tiles per node block

_KER_CACHE = {}


def build_G():
    """Init embeddings via multi-hot matmuls: x0T, x5T, x6T (feature-major)."""
    nc = bacc.Bacc()
    atab = nc.dram_tensor("atab", [5, P, P], BF16, kind="ExternalInput")
    mh = nc.dram_tensor("mh", [5, P, NDP], BF16, kind="ExternalInput")
    ctab = nc.dram_tensor("ctab", [16, P], BF16, kind="ExternalInput")
    mh5 = nc.dram_tensor("mh5", [16, NP5P], BF16, kind="ExternalInput")
    mh6 = nc.dram_tensor("mh6", [16, NP6P], BF16, kind="ExternalInput")
    x0T = nc.dram_tensor("x0T", [P, NDP], BF16, kind="ExternalOutput")
    x5T = nc.dram_tensor("x5T", [P, NP5P], BF16, kind="ExternalOutput")
    x6T = nc.dram_tensor("x6T", [P, NP6P], BF16, kind="ExternalOutput")
    with tile.TileContext(nc) as tc:
        with (
            tc.tile_pool(name="cons", bufs=1) as cons,
            tc.tile_pool(name="sb", bufs=3) as sb,
            tc.tile_pool(name="ps", bufs=2, space="PSUM") as ps,
        ):
            at = [cons.tile([P, P], BF16, tag=f"at{t}", name=f"at{t}") for t in range(5)]
            for t in range(5):
                nc.sync.dma_start(out=at[t][:], in_=atab[t])
            ct = cons.tile([16, P], BF16)
            nc.sync.dma_start(out=ct[:], in_=ctab[:])
            # atoms: 4-block chunks
            for g in range(NB // 4):
                mt = sb.tile([P, 5, 4 * P], BF16, tag="mt")
                nc.scalar.dma_start(
                    out=mt[:], in_=mh[:, :, g * 4 * P:(g + 1) * 4 * P].rearrange("t p h -> p t h"))
                xp = ps.tile([P, 4 * P], F32, tag="xp")
                for t in range(5):
                    nc.tensor.matmul(out=xp[:], lhsT=at[t][:], rhs=mt[:, t, :],
                                     start=(t == 0), stop=(t == 4))
                xs = sb.tile([P, 4 * P], BF16, tag="xs")
                nc.vector.tensor_copy(out=xs[:], in_=xp[:])
                nc.sync.dma_start(out=x0T[:, g * 4 * P:(g + 1) * 4 * P], in_=xs[:])
            # cycles: per 640/768-block, halves
            for k, nblk, mhk, xko in ((5, D5B, mh5, x5T), (6, D6B, mh6, x6T)):
                hp = 64 * k
                for b in range(nblk):
                    m5 = sb.tile([16, 2 * hp], BF16, tag="m5")
                    nc.gpsimd.dma_start(out=m5[:], in_=mhk[:, b * 2 * hp:(b + 1) * 2 * hp])
                    xo = sb.tile([P, 2 * hp], BF16, tag="xo5")
                    for hh in range(2):
                        cp = ps.tile([P, hp], F32, tag="cp")
                        nc.tensor.matmul(out=cp[:], lhsT=ct[:], rhs=m5[:, hh * hp:(hh + 1) * hp],
                                         start=True, stop=True)
                        nc.vector.tensor_copy(out=xo[:, hh * hp:(hh + 1) * hp], in_=cp[:])
                    nc.sync.dma_start(out=xko[:, b * 2 * hp:(b + 1) * 2 * hp], in_=xo[:])
    nc.compile()
    return nc


def build_A():
    """Edge agg (pre-gathered) + GEMM1 (x-term folded) + BN1 stats."""
    nc = bacc.Bacc()
    sg = nc.dram_tensor("sg", [P, NET // 2, 2 * P], BF16, kind="ExternalInput")
    ohb = nc.dram_tensor("ohb", [P, NET // 2, 2 * P], BF16, kind="ExternalInput")
    xT = nc.dram_tensor("xT", [P, NDP], BF16, kind="ExternalInput")
    w1 = nc.dram_tensor("w1", [P, 2 * P], BF16, kind="ExternalInput")
    w1s = nc.dram_tensor("w1s", [P, 2 * P], BF16, kind="ExternalInput")
    t1T = nc.dram_tensor("t1T", [2 * P, NDP], BF16, kind="ExternalOutput")
    bstat = nc.dram_tensor("bstat", [2, P, 2], F32, kind="ExternalOutput")
    NG = NB // 4  # 26 groups of 4 blocks
    with tile.TileContext(nc) as tc:
        with (
            tc.tile_pool(name="cons", bufs=1) as cons,
            tc.tile_pool(name="xr", bufs=1) as xr,
            tc.tile_pool(name="sb", bufs=4) as sb,
            tc.tile_pool(name="st", bufs=1) as st,
            tc.tile_pool(name="ps", bufs=2, space="PSUM") as ps,
            tc.tile_pool(name="pt", bufs=2, space="PSUM") as pt,
        ):
            w1t = cons.tile([P, 2 * P], BF16)
            nc.sync.dma_start(out=w1t[:], in_=w1[:])
            w1st = cons.tile([P, 2 * P], BF16)
            nc.sync.dma_start(out=w1st[:], in_=w1s[:])
            xt = xr.tile([P, NDP], BF16)
            for j in range(13):
                nc.scalar.dma_start(out=xt[:, j * 1024:(j + 1) * 1024],
                                    in_=xT[:, j * 1024:(j + 1) * 1024])
            stat = [st.tile([P, NG, 6], F32, tag=f"sst{h}", name=f"sst{h}") for h in range(2)]
            for g in range(NG):
                agg4 = sb.tile([P, 4 * P], BF16, tag="agg4")
                for bb2 in range(2):
                    b2 = 2 * g + bb2  # 2-block chunk index
                    sgt = sb.tile([P, 4, 2, P], BF16, tag="sgt")
                    nc.gpsimd.dma_start(
                        out=sgt[:],
                        in_=sg[:, b2 * 4:(b2 + 1) * 4, :].rearrange("p t (i h) -> p t i h", i=2))
                    sl = sb.tile([P, 4, 2, P], BF16, tag="sl")
                    nc.scalar.activation(out=sl[:].rearrange("p t i h -> p (t i h)"),
                                         in_=sgt[:].rearrange("p t i h -> p (t i h)"), func=RELU)
                    oh = sb.tile([P, 4, 2, P], BF16, tag="oh")
                    nc.sync.dma_start(
                        out=oh[:],
                        in_=ohb[:, b2 * 4:(b2 + 1) * 4, :].rearrange("p t (i h) -> p t i h", i=2))
                    for bb in range(2):
                        aggT = ps.tile([P, P], F32, tag="aggT")
                        for t in range(KE):
                            tt = bb * 2 + t // 2
                            i = t % 2
                            nc.tensor.matmul(out=aggT[:], lhsT=sl[:, tt, i, :],
                                             rhs=oh[:, tt, i, :],
                                             start=(t == 0), stop=(t == KE - 1))
                        nc.scalar.activation(out=agg4[:, (2 * bb2 + bb) * P:(2 * bb2 + bb + 1) * P],
                                             in_=aggT[:], func=COPY)
                for half in range(2):
                    t1p = pt.tile([P, 4 * P], F32, tag=f"t1p{half}")
                    nc.tensor.matmul(out=t1p[:], lhsT=w1t[:, half * P:(half + 1) * P],
                                     rhs=agg4[:], start=True, stop=False)
                    nc.tensor.matmul(out=t1p[:], lhsT=w1st[:, half * P:(half + 1) * P],
                                     rhs=xt[:, g * 4 * P:(g + 1) * 4 * P], start=False, stop=True)
                    t1s = sb.tile([P, 4 * P], BF16, tag=f"t1s{half}")
                    nc.vector.tensor_copy(out=t1s[:], in_=t1p[:])
                    nc.vector.bn_stats(out=stat[half][:, g, :], in_=t1s[:])
                    nc.sync.dma_start(out=t1T[half * P:(half + 1) * P, g * 4 * P:(g + 1) * 4 * P],
                                      in_=t1s[:])
            for half in range(2):
                mv = sb.tile([P, 2], F32, tag="mv")
                nc.vector.bn_aggr(out=mv[:], in_=stat[half][:])
                nc.sync.dma_start(out=bstat[half], in_=mv[:])
    nc.compile()
    return nc


def build_B():
    """t2 = relu(t1*a1+b1); hT = w2-chain; BN2 stats."""
    nc = bacc.Bacc()
    t1T = nc.dram_tensor("t1T", [2 * P, NDP], BF16, kind="ExternalInput")
    ab1 = nc.dram_tensor("ab1", [2, 2, P, 1], F32, kind="ExternalInput")
    gw2 = nc.dram_tensor("gw2", [2 * P, P], BF16, kind="ExternalInput")
    hT = nc.dram_tensor("hT", [P, NDP], BF16, kind="ExternalOutput")
    bstat = nc.dram_tensor("bstat", [P, 2], F32, kind="ExternalOutput")
    NG = NB // 4
    with tile.TileContext(nc) as tc:
        with (
            tc.tile_pool(name="cons", bufs=1) as cons,
            tc.tile_pool(name="sb", bufs=4) as sb,
            tc.tile_pool(name="st", bufs=1) as st,
            tc.tile_pool(name="ps", bufs=2, space="PSUM") as ps,
        ):
            w2 = [cons.tile([P, P], BF16, tag=f"w2{h}", name=f"w2{h}") for h in range(2)]
            a1 = [cons.tile([P, 1], F32, tag=f"a{h}", name=f"a1_{h}") for h in range(2)]
            b1 = [cons.tile([P, 1], F32, tag=f"b{h}", name=f"b1_{h}") for h in range(2)]
            for h in range(2):
                nc.sync.dma_start(out=w2[h][:], in_=gw2[h * P:(h + 1) * P, :])
                nc.sync.dma_start(out=a1[h][:], in_=ab1[h, 0])
                nc.sync.dma_start(out=b1[h][:], in_=ab1[h, 1])
            stat = st.tile([P, NG, 6], F32)
            for g in range(NG):
                hp = ps.tile([P, 4 * P], F32, tag="hp")
                for half in range(2):
                    t1s = sb.tile([P, 4 * P], BF16, tag=f"t1s{half}")
                    (nc.scalar if half == 0 else nc.gpsimd).dma_start(
                        out=t1s[:], in_=t1T[half * P:(half + 1) * P,
                                            g * 4 * P:(g + 1) * 4 * P])
                    t2s = sb.tile([P, 4 * P], BF16, tag=f"t2s{half}")
                    nc.scalar.activation(out=t2s[:], in_=t1s[:], func=RELU,
                                         bias=b1[half][:], scale=a1[half][:])
                    nc.tensor.matmul(out=hp[:], lhsT=w2[half][:], rhs=t2s[:],
                                     start=(half == 0), stop=(half == 1))
                hs = sb.tile([P, 4 * P], BF16, tag="hs")
                nc.vector.tensor_copy(out=hs[:], in_=hp[:])
                nc.vector.bn_stats(out=stat[:, g, :], in_=hs[:])
                nc.sync.dma_start(out=hT[:, g * 4 * P:(g + 1) * 4 * P], in_=hs[:])
            mv = sb.tile([P, 2], F32, tag="mv")
            nc.vector.bn_aggr(out=mv[:], in_=stat[:])
            nc.sync.dma_start(out=bstat[:], in_=mv[:])
    nc.compile()
    return nc


def build_CD():
    """BN2-apply (xT out) + a2c mixes + cyclic path blocks for c5 and c6."""
    nc = bacc.Bacc()
    hT = nc.dram_tensor("hT", [P, NDP], BF16, kind="ExternalInput")
    ab2 = nc.dram_tensor("ab2", [2, P, 1], F32, kind="ExternalInput")
    z5g = nc.dram_tensor("z5g", [P, NP5P], BF16, kind="ExternalInput")
    z6g = nc.dram_tensor("z6g", [P, NP6P], BF16, kind="ExternalInput")
    x5T = nc.dram_tensor("x5T", [P, NP5P], BF16, kind="ExternalInput")
    x6T = nc.dram_tensor("x6T", [P, NP6P], BF16, kind="ExternalInput")
    aw5 = nc.dram_tensor("aw5", [P, P], BF16, kind="ExternalInput")
    ab5 = nc.dram_tensor("ab5", [P, 1], F32, kind="ExternalInput")
    aw6 = nc.dram_tensor("aw6", [P, P], BF16, kind="ExternalInput")
    ab6 = nc.dram_tensor("ab6", [P, 1], F32, kind="ExternalInput")
    pw5 = nc.dram_tensor("pw5", [3, P, P], BF16, kind="ExternalInput")
    pb5 = nc.dram_tensor("pb5", [P, 1], F32, kind="ExternalInput")
    pw6 = nc.dram_tensor("pw6", [3, P, P], BF16, kind="ExternalInput")
    pb6 = nc.dram_tensor("pb6", [P, 1], F32, kind="ExternalInput")
    xT = nc.dram_tensor("xT", [P, NDP], BF16, kind="ExternalOutput")
    x5To = nc.dram_tensor("x5To", [P, NP5P], BF16, kind="ExternalOutput")
    x6To = nc.dram_tensor("x6To", [P, NP6P], BF16, kind="ExternalOutput")
    with tile.TileContext(nc) as tc:
        with (
            tc.tile_pool(name="cons", bufs=1) as cons,
            tc.tile_pool(name="sb", bufs=4) as sb,
            tc.tile_pool(name="ps", bufs=3, space="PSUM") as ps,
        ):
            a2 = cons.tile([P, 1], F32)
            nc.sync.dma_start(out=a2[:], in_=ab2[0])
            b2 = cons.tile([P, 1], F32)
            nc.sync.dma_start(out=b2[:], in_=ab2[1])
            awt = {5: cons.tile([P, P], BF16, name="aw5t"), 6: cons.tile([P, P], BF16, name="aw6t")}
            abt = {5: cons.tile([P, 1], F32, name="ab5t"), 6: cons.tile([P, 1], F32, name="ab6t")}
            pwt = {5: [cons.tile([P, P], BF16, name=f"pw5{s}") for s in range(3)],
                   6: [cons.tile([P, P], BF16, name=f"pw6{s}") for s in range(3)]}
            pbt = {5: cons.tile([P, 1], F32, name="pb5t"), 6: cons.tile([P, 1], F32, name="pb6t")}
            for k, aws, abs_, pws, pbs in ((5, aw5, ab5, pw5, pb5), (6, aw6, ab6, pw6, pb6)):
                nc.sync.dma_start(out=awt[k][:], in_=aws[:])
                nc.sync.dma_start(out=abt[k][:], in_=abs_[:])
                for s in range(3):
                    nc.sync.dma_start(out=pwt[k][s][:], in_=pws[s])
                nc.sync.dma_start(out=pbt[k][:], in_=pbs[:])
            # C part: xT = relu(a2*hT + b2)
            for j in range(13):
                hs = sb.tile([P, 1024], BF16, tag="hs")
                nc.scalar.dma_start(out=hs[:], in_=hT[:, j * 1024:(j + 1) * 1024])
                xs = sb.tile([P, 1024], BF16, tag="xs")
                nc.scalar.activation(out=xs[:], in_=hs[:], func=RELU,
                                     bias=b2[:], scale=a2[:])
                nc.sync.dma_start(out=xT[:, j * 1024:(j + 1) * 1024], in_=xs[:])
            # D part per k: halo-layout conv, bf16 state
            for k, nblk, zg, xk, xko in ((5, D5B, z5g, x5T, x5To), (6, D6B, z6g, x6T, x6To)):
                BPOS = P * k
                HP = 64 * k
                for blk in range(nblk):
                    zt = sb.tile([P, BPOS], BF16, tag="zt")
                    nc.gpsimd.dma_start(out=zt[:], in_=zg[:, blk * BPOS:(blk + 1) * BPOS])
                    x5b = sb.tile([P, BPOS], BF16, tag="x5b")
                    nc.gpsimd.dma_start(out=x5b[:], in_=xk[:, blk * BPOS:(blk + 1) * BPOS])
                    xo = sb.tile([P, BPOS], BF16, tag="xo")
                    for hh in range(2):
                        zb = sb.tile([P, HP], BF16, tag="zb")
                        nc.scalar.activation(out=zb[:], in_=zt[:, hh * HP:(hh + 1) * HP],
                                             func=RELU, bias=b2[:], scale=a2[:])
                        rp = ps.tile([P, HP], F32, tag="rp")
                        nc.tensor.matmul(out=rp[:], lhsT=awt[k][:], rhs=zb[:],
                                         start=True, stop=True)
                        rs = sb.tile([P, HP], F32, tag="rs")
                        nc.scalar.activation(out=rs[:], in_=rp[:], func=RELU, bias=abt[k][:])
                        xv3 = x5b[:, hh * HP:(hh + 1) * HP].rearrange("h (c j) -> h c j", j=k)
                        xch = sb.tile([P, 64, k + 2], BF16, tag="xch")
                        nc.vector.tensor_add(out=xch[:, :, 1:k + 1], in0=xv3,
                                             in1=rs[:].rearrange("h (c j) -> h c j", j=k))
                        nc.vector.tensor_copy(out=xch[:, :, 0:1], in_=xch[:, :, k:k + 1])
                        nc.vector.tensor_copy(out=xch[:, :, k + 1:k + 2], in_=xch[:, :, 1:2])
                        cvp = ps.tile([P, HP], F32, tag="cvp")
                        for s in range(3):
                            nc.tensor.matmul(out=cvp[:], lhsT=pwt[k][s][:],
                                             rhs=xch[:, :, s:s + k], start=(s == 0), stop=(s == 2))
                        cvr = sb.tile([P, HP], F32, tag="cvr")
                        nc.scalar.activation(out=cvr[:], in_=cvp[:], func=RELU, bias=pbt[k][:])
                        nc.vector.tensor_add(
                            out=xo[:, hh * HP:(hh + 1) * HP].rearrange("h (c j) -> h c j", j=k),
                            in0=xch[:, :, 1:k + 1], in1=cvr[:].rearrange("h (c j) -> h c j", j=k))
                    nc.sync.dma_start(out=xko[:, blk * BPOS:(blk + 1) * BPOS], in_=xo[:])
    nc.compile()
    return nc


def build_E(readout):
    """c2a: seg-mean (pre-gathered, pre-scaled) + linear + relu + residual.
    readout=True: fold the graph readout (F) in instead of storing xT'."""
    nc = bacc.Bacc()
    xT = nc.dram_tensor("xT", [P, NDP], BF16, kind="ExternalInput")
    u5g = nc.dram_tensor("u5g", [P, NB * K5 // 2, 2 * P], BF16, kind="ExternalInput")
    u6g = nc.dram_tensor("u6g", [P, NB * K6 // 2, 2 * P], BF16, kind="ExternalInput")
    drel5 = nc.dram_tensor("drel5", [P, NB * K5], F32, kind="ExternalInput")
    drel6 = nc.dram_tensor("drel6", [P, NB * K6], F32, kind="ExternalInput")
    iotaf = nc.dram_tensor("iotaf", [P, 8 * P], F32, kind="ExternalInput")
    w5 = nc.dram_tensor("w5", [P, P], BF16, kind="ExternalInput")
    b5 = nc.dram_tensor("b5", [P, 1], F32, kind="ExternalInput")
    w6 = nc.dram_tensor("w6", [P, P], BF16, kind="ExternalInput")
    b6 = nc.dram_tensor("b6", [P, 1], F32, kind="ExternalInput")
    if readout:
        grel = nc.dram_tensor("grel", [P, NB], F32, kind="ExternalInput")
        cig = nc.dram_tensor("cig", [P, GPC], F32, kind="ExternalInput")
        alw = nc.dram_tensor("alw", [P, P], F32, kind="ExternalInput")
        alb = nc.dram_tensor("alb", [P, 1], F32, kind="ExternalInput")
        linw = nc.dram_tensor("linw", [P, 1], F32, kind="ExternalInput")
        linb = nc.dram_tensor("linb", [1, 1], F32, kind="ExternalInput")
        y = nc.dram_tensor("y", [1, GPC], F32, kind="ExternalOutput")
    else:
        xTo = nc.dram_tensor("xTo", [P, NDP], BF16, kind="ExternalOutput")
    with tile.TileContext(nc) as tc:
        with (
            tc.tile_pool(name="cons", bufs=1) as cons,
            tc.tile_pool(name="sb", bufs=4) as sb,
            tc.tile_pool(name="psa", bufs=(1 if readout else 2), space="PSUM") as psa,
            tc.tile_pool(name="psb", bufs=(3 if readout else 2), space="PSUM") as psb,
            tc.tile_pool(name="psg", bufs=1, space="PSUM") as psg,
        ):
            iot8 = cons.tile([P, 8, P], F32)
            nc.sync.dma_start(out=iot8[:], in_=iotaf[:].rearrange("p (k h) -> p k h", k=8))
            dr5 = cons.tile([P, NB * K5], F32)
            nc.sync.dma_start(out=dr5[:], in_=drel5[:])
            dr6 = cons.tile([P, NB * K6], F32)
            nc.sync.dma_start(out=dr6[:], in_=drel6[:])
            wt = {5: cons.tile([P, P], BF16, name="w5t"), 6: cons.tile([P, P], BF16, name="w6t")}
            bt = {5: cons.tile([P, 1], F32, name="b5t"), 6: cons.tile([P, 1], F32, name="b6t")}
            nc.sync.dma_start(out=wt[5][:], in_=w5[:])
            nc.sync.dma_start(out=bt[5][:], in_=b5[:])
            nc.sync.dma_start(out=wt[6][:], in_=w6[:])
            nc.sync.dma_start(out=bt[6][:], in_=b6[:])
            if readout:
                ident = cons.tile([P, P], F32)
                make_identity(nc, ident[:])
                gr = cons.tile([P, NB], F32)
                nc.sync.dma_start(out=gr[:], in_=grel[:])
                cigt = cons.tile([P, GPC], F32)
                nc.sync.dma_start(out=cigt[:], in_=cig[:])
                alwt = cons.tile([P, P], F32)
                nc.sync.dma_start(out=alwt[:], in_=alw[:])
                albt = cons.tile([P, 1], F32)
                nc.sync.dma_start(out=albt[:], in_=alb[:])
                linwt = cons.tile([P, 1], F32)
                nc.sync.dma_start(out=linwt[:], in_=linw[:])
                linbt = cons.tile([1, 1], F32)
                nc.sync.dma_start(out=linbt[:], in_=linb[:])
                xgT = psg.tile([P, GPC], F32)
            for b in range(NB):
                if b % 4 == 0:
                    u5t = sb.tile([P, 4, 2, P], BF16, tag="u5t")
                    nc.scalar.dma_start(
                        out=u5t[:],
                        in_=u5g[:, b:b + 4, :].rearrange("p t (i h) -> p t i h", i=2))
                    xb = sb.tile([P, 4 * P], BF16, tag="xb")
                    nc.gpsimd.dma_start(out=xb[:], in_=xT[:, b * P:(b + 4) * P])
                    if not readout:
                        xno = sb.tile([P, 4 * P], BF16, tag="xno")
                if b % 2 == 0:
                    u6t = sb.tile([P, 3, 2, P], BF16, tag="u6t")
                    nc.gpsimd.dma_start(
                        out=u6t[:],
                        in_=u6g[:, b * 3 // 2:b * 3 // 2 + 3, :].rearrange(
                            "p t (i h) -> p t i h", i=2))
                rr = {}
                for k, K, ut, drk in ((5, K5, u5t, dr5), (6, K6, u6t, dr6)):
                    uT = psa.tile([P, P], F32, tag=f"uT{k}")
                    oh = sb.tile([P, K, P], BF16, tag=f"oh{k}")
                    nc.vector.tensor_tensor(
                        out=oh[:], in0=drk[:, b * K:(b + 1) * K].to_broadcast([P, K, P]),
                        in1=iot8[:, 0:K, :], op=EQ)
                    for t in range(K):
                        lt = (b % 4) * K + t if k == 5 else (b % 2) * K + t
                        nc.tensor.matmul(out=uT[:], lhsT=ut[:, lt // 2, lt % 2, :],
                                         rhs=oh[:, t, :],
                                         start=(t == 0), stop=(t == K - 1))
                    us = sb.tile([P, P], BF16, tag=f"us{k}")
                    if k == 5:
                        nc.vector.tensor_copy(out=us[:], in_=uT[:])
                    else:
                        nc.scalar.activation(out=us[:], in_=uT[:], func=COPY)
                    rp = psb.tile([P, P], F32, tag="rp")
                    nc.tensor.matmul(out=rp[:], lhsT=wt[k][:], rhs=us[:], start=True, stop=True)
                    rs = sb.tile([P, P], BF16, tag=f"rs{k}")
                    nc.scalar.activation(out=rs[:], in_=rp[:], func=RELU, bias=bt[k][:])
                    rr[k] = rs
                xn = sb.tile([P, P], BF16, tag="xn")
                nc.gpsimd.tensor_add(out=xn[:], in0=xb[:, (b % 4) * P:(b % 4 + 1) * P],
                                     in1=rr[5][:])
                if readout:
                    nc.vector.tensor_add(out=xn[:], in0=xn[:], in1=rr[6][:])
                    xf = sb.tile([P, P], F32, tag="xf")
                    nc.scalar.activation(out=xf[:], in_=xn[:], func=COPY)
                    tp = psb.tile([P, P], F32, tag="rp")
                    nc.tensor.transpose(out=tp[:], in_=xf[:], identity=ident[:])
                    xfT = sb.tile([P, P], F32, tag="xfT")
                    nc.vector.tensor_copy(out=xfT[:], in_=tp[:])
                    ohg = sb.tile([P, GPC], F32, tag="ohg")
                    nc.vector.tensor_tensor(out=ohg[:], in0=gr[:, b:b + 1].to_broadcast([P, GPC]),
                                            in1=iot8[:, 0, :GPC], op=EQ)
                    nc.tensor.matmul(out=xgT[:], lhsT=xfT[:], rhs=ohg[:],
                                     start=(b == 0), stop=(b == NB - 1))
                else:
                    nc.vector.tensor_add(out=xno[:, (b % 4) * P:(b % 4 + 1) * P],
                                         in0=xn[:], in1=rr[6][:])
                    if b % 4 == 3:
                        nc.sync.dma_start(out=xTo[:, (b - 3) * P:(b + 1) * P], in_=xno[:])
            if readout:
                xg = sb.tile([P, GPC], F32, tag="xg")
                nc.vector.tensor_mul(out=xg[:], in0=xgT[:], in1=cigt[:])
                ap = psg.tile([P, GPC], F32, tag="ap")
                nc.tensor.matmul(out=ap[:], lhsT=alwt[:], rhs=xg[:], start=True, stop=True)
                av = sb.tile([P, GPC], F32, tag="av")
                nc.scalar.activation(out=av[:], in_=ap[:], func=RELU, bias=albt[:])
                yp = psg.tile([1, GPC], F32, tag="yp")
                nc.tensor.matmul(out=yp[:], lhsT=linwt[:], rhs=av[:], start=True, stop=True)
                ys = sb.tile([1, GPC], F32, tag="ys")
                nc.vector.tensor_scalar_add(out=ys[:], in0=yp[:], scalar1=linbt[:])
                nc.sync.dma_start(out=y[:], in_=ys[:])
    nc.compile()
    return nc


def get_kernels():
    if "G" not in _KER_CACHE:
        _KER_CACHE.update(G=build_G(), A=build_A(), B=build_B(), CD=build_CD(),
                          E=build_E(False), E3=build_E(True))
    return _KER_CACHE


# ---------------------------------------------------------------- host glue

def slotmajor(vals, ntiles):
    """[ntiles*128, H] -> [128, ntiles//2, 2*H] bf16 (pair-interleaved)."""
    return np.ascontiguousarray(
        vals.reshape(ntiles // 2, 2, P, H).transpose(2, 0, 1, 3).reshape(
            P, ntiles // 2, 2 * H)).astype(NPBF)


class Prep:
    """Per-core layer-invariant index prep."""

    def __init__(self, x_atom, edge_index, edge_attr, batch, xc5, xc6, r5, r6):
        self.iotaf = np.tile(np.arange(P, dtype=np.float32)[None, :], (P, 8))
        core_of_node = (batch // GPC).astype(np.int64)
        self.node_lo = np.searchsorted(batch, np.arange(NC) * GPC)
        self.node_hi = np.searchsorted(batch, np.arange(NC) * GPC + GPC)
        self.nd = self.node_hi - self.node_lo
        assert self.nd.max() <= NDP
        src, dst = edge_index[0], edge_index[1]
        combo = (edge_attr[:, 0] * (BV * BV) + edge_attr[:, 1] * BV + edge_attr[:, 2])
        self.cores = []
        for c in range(NC):
            d = {}
            lo, hi, nd = self.node_lo[c], self.node_hi[c], self.nd[c]
            # ---- edge slots grouped by dst block
            em = np.where(core_of_node[dst] == c)[0]
            eblk = (dst[em] - lo) // P
            order = np.argsort(eblk, kind="stable")
            em = em[order]; eblk = eblk[order]
            cnt = np.bincount(eblk, minlength=NB)
            assert cnt.max() <= KE * P, f"edge block overflow {cnt.max()}"
            nslot = NET * P
            slot_src = np.zeros(nslot, dtype=np.int64)
            slot_ea = np.full(nslot, 512, dtype=np.int64)
            slot_dr = np.full(nslot, 255.0, dtype=np.float32)
            starts = np.concatenate([[0], np.cumsum(cnt)])
            for b in range(NB):
                sl = b * KE * P
                e = em[starts[b]:starts[b + 1]]
                slot_src[sl:sl + len(e)] = src[e]
                slot_ea[sl:sl + len(e)] = combo[e]
                slot_dr[sl:sl + len(e)] = (dst[e] - lo - b * P).astype(np.float32)
            d["slot_src"] = slot_src
            d["slot_ea"] = slot_ea
            dstrel = np.ascontiguousarray(slot_dr.reshape(NET, P).T)
            d["ohb"] = (dstrel[:, :, None]
                        == np.arange(P, dtype=np.float32)[None, None, :]).astype(
                            NPBF).reshape(P, NET // 2, 2 * P)
            # ---- z rows (a2c sources): global node ids per local cycle position
            for kk, npos, nposp, rows_all in ((5, NP5, NP5P, r5), (6, NP6, NP6P, r6)):
                rp = np.zeros(nposp, dtype=np.int64)
                rp[:npos] = rows_all[c * npos:(c + 1) * npos]
                d[f"z{kk}rows"] = rp
                d[f"z{kk}mask"] = npos
            # ---- u slots (c2a): positions targeting this core's nodes
            cnt5 = np.bincount(r5, minlength=N).astype(np.float32)
            cnt6 = np.bincount(r6, minlength=N).astype(np.float32)
            for kk, rows_all, K, cnt_node in ((5, r5, K5, cnt5), (6, r6, K6, cnt6)):
                pm = np.where(core_of_node[rows_all] == c)[0]
                tblk = (rows_all[pm] - lo) // P
                order = np.argsort(tblk, kind="stable")
                pm = pm[order]; tblk = tblk[order]
                cntb = np.bincount(tblk, minlength=NB)
                assert cntb.max() <= K * P, f"u{kk} block overflow {cntb.max()}"
                nslot = NB * K * P
                slot_pos = np.zeros(nslot, dtype=np.int64)
                slot_dr = np.full(nslot, 255.0, dtype=np.float32)
                slot_cs = np.zeros(nslot, dtype=np.float32)
                cinv = 1.0 / np.maximum(cnt_node, 1.0)
                st = np.concatenate([[0], np.cumsum(cntb)])
                for b in range(NB):
                    sl = b * K * P
                    pp = pm[st[b]:st[b + 1]]
                    slot_pos[sl:sl + len(pp)] = pp
                    slot_dr[sl:sl + len(pp)] = (rows_all[pp] - lo - b * P).astype(np.float32)
                    slot_cs[sl:sl + len(pp)] = cinv[rows_all[pp]]
                d[f"u{kk}pos"] = slot_pos
                d[f"u{kk}cs"] = slot_cs[:, None]
                d[f"drel{kk}"] = np.ascontiguousarray(slot_dr.reshape(NB * K, P).T)
            # ---- init multi-hots
            mh = np.zeros((640, NDP), dtype=np.float32)
            colr = np.arange(nd)
            for f in range(AF):
                mh[f * AV + x_atom[lo:hi, f], colr] = 1.0
            d["mh"] = np.ascontiguousarray(mh.reshape(5, P, NDP)).astype(NPBF)
            mh5 = np.zeros((16, NP5P), dtype=np.float32)
            mh5[xc5[c * NP5:(c + 1) * NP5], np.arange(NP5)] = 1.0
            d["mh5"] = mh5.astype(NPBF)
            mh6 = np.zeros((16, NP6P), dtype=np.float32)
            mh6[4 + xc6[c * NP6:(c + 1) * NP6], np.arange(NP6)] = 1.0
            d["mh6"] = mh6.astype(NPBF)
            # ---- readout
            grel = np.full((NB * P,), 255.0, dtype=np.float32)
            grel[:nd] = (batch[lo:hi] - c * GPC).astype(np.float32)
            d["grel"] = np.ascontiguousarray(grel.reshape(NB, P).T)
            gsz = np.bincount(batch, minlength=G).astype(np.float32)[c * GPC:(c + 1) * GPC]
            d["cig"] = np.tile(1.0 / np.maximum(gsz, 1.0)[None, :], (P, 1))
            self.cores.append(d)


def _run(nc, in_maps, trace=False):
    return run_bass_kernel_spmd(nc, in_maps, core_ids=list(range(NC)), trace=trace)


_EXEC_NS = []  # exec_time_ns per launch when tracing


def kernel(**inputs):
    inp = {k: np.asarray(v) for k, v in inputs.items()}
    x_atom = inp["x_atom"].astype(np.int64)
    edge_index = inp["edge_index"].astype(np.int64)
    edge_attr = inp["edge_attr"].astype(np.int64)
    batch = inp["batch"].astype(np.int64)
    xc5 = inp["xc5"].astype(np.int64); xc6 = inp["xc6"].astype(np.int64)
    r5 = inp["a2c5_row"].astype(np.int64); r6 = inp["a2c6_row"].astype(np.int64)
    f32 = lambda k: inp[k].astype(np.float32)
    atom_emb = f32("atom_emb"); bond_emb = f32("bond_emb")
    cyc5 = f32("cyc_emb5"); cyc6 = f32("cyc_emb6"); eps = f32("gine_eps")
    gw1 = f32("gw1"); gbn_g = f32("gbn_g"); gbn_b = f32("gbn_b")
    gw2 = f32("gw2"); bn_g = f32("bn_g"); bn_b = f32("bn_b")
    trace = bool(int(__import__("os").environ.get("CYC_TRACE", "0")))

    prep = Prep(x_atom, edge_index, edge_attr, batch, xc5, xc6, r5, r6)
    ks = get_kernels()
    _EXEC_NS.clear()

    def run(name, maps):
        res = _run(ks[name], maps, trace=trace)
        if trace and res.exec_time_ns is not None:
            _EXEC_NS.append((name, res.exec_time_ns))
        return res.results

    # ---- init embeddings
    atab = np.zeros((640, H), np.float32)
    atab[:AF * AV] = atom_emb.reshape(AF * AV, H)
    atab = np.ascontiguousarray(atab.reshape(5, P, H)).astype(NPBF)
    ctab = np.zeros((16, H), np.float32)
    ctab[0:4] = cyc5; ctab[4:8] = cyc6
    ctab = ctab.astype(NPBF)
    rG = run("G", [{"atab": atab, "ctab": ctab, "mh": d["mh"],
                    "mh5": d["mh5"], "mh6": d["mh6"]} for d in prep.cores])
    x_full = np.concatenate([
        np.asarray(rG[c]["x0T"]).astype(np.float32).T[:prep.nd[c]] for c in range(NC)])
    x5loc = [np.asarray(rG[c]["x5T"]) for c in range(NC)]
    x6loc = [np.asarray(rG[c]["x6T"]) for c in range(NC)]

    def xT_of(xf):
        """x_full [N,H] -> per-core zero-padded feature-major bf16 [P, NDP]."""
        outs = []
        for c in range(NC):
            m = np.zeros((NDP, H), np.float32)
            m[:prep.nd[c]] = xf[prep.node_lo[c]:prep.node_hi[c]]
            outs.append(np.ascontiguousarray(m.T).astype(NPBF))
        return outs

    for i in range(L):
        xTs = xT_of(x_full)
        be = bond_emb[i]
        combos = np.arange(BV ** 3)
        etab = (be[0][combos // (BV * BV)] + be[1][(combos // BV) % BV] + be[2][combos % BV])
        etab = np.concatenate([etab, np.zeros((1, H), np.float32)])
        w1 = gw1[i].astype(NPBF)
        w1s = (gw1[i] * (1.0 + eps[i])).astype(NPBF)
        # ---- A
        mapsA = []
        for c, d in enumerate(prep.cores):
            vals = x_full[d["slot_src"]] + etab[d["slot_ea"]]
            mapsA.append({"sg": slotmajor(vals, NET), "ohb": d["ohb"],
                          "xT": xTs[c], "w1": w1, "w1s": w1s})
        rA = run("A", mapsA)
        m = np.stack([np.concatenate([rA[c]["bstat"][0, :, 0], rA[c]["bstat"][1, :, 0]])
                      for c in range(NC)]).astype(np.float64)
        v = np.stack([np.concatenate([rA[c]["bstat"][0, :, 1], rA[c]["bstat"][1, :, 1]])
                      for c in range(NC)]).astype(np.float64)
        tot = m.sum(0) * NDP
        tot2 = (v + m ** 2).sum(0) * NDP
        m1 = tot / N
        v1 = tot2 / N - m1 ** 2
        a1 = (gbn_g[i] / np.sqrt(v1 + BN_EPS)).astype(np.float32)
        b1 = (gbn_b[i] - a1 * m1).astype(np.float32)
        ab1 = np.stack([np.stack([a1[h * P:(h + 1) * P, None], b1[h * P:(h + 1) * P, None]])
                        for h in range(2)])
        # ---- B
        rB = run("B", [{"t1T": rA[c]["t1T"], "ab1": ab1, "gw2": gw2[i].astype(NPBF)}
                       for c in range(NC)])
        m2 = np.stack([rB[c]["bstat"][:, 0] for c in range(NC)]).astype(np.float64)
        v2 = np.stack([rB[c]["bstat"][:, 1] for c in range(NC)]).astype(np.float64)
        hpad = (np.maximum(b1, 0.0).astype(np.float64) @ gw2[i].astype(np.float64))
        npad = NC * NDP - N
        tot = m2.sum(0) * NDP - npad * hpad
        tot2 = (v2 + m2 ** 2).sum(0) * NDP - npad * hpad ** 2
        m2g = tot / N
        v2g = tot2 / N - m2g ** 2
        a2 = (bn_g[i] / np.sqrt(v2g + BN_EPS)).astype(np.float32)
        b2 = (bn_b[i] - a2 * m2g).astype(np.float32)
        ab2 = np.stack([a2[:, None], b2[:, None]])
        # ---- CD
        h_full = np.concatenate([
            np.asarray(rB[c]["hT"]).astype(np.float32).T[:prep.nd[c]] for c in range(NC)])
        mapsCD = []
        for c, d in enumerate(prep.cores):
            z5 = np.ascontiguousarray(h_full[d["z5rows"]].T).astype(NPBF)
            z6 = np.ascontiguousarray(h_full[d["z6rows"]].T).astype(NPBF)
            mapsCD.append({"hT": rB[c]["hT"], "ab2": ab2, "z5g": z5, "z6g": z6,
                           "x5T": x5loc[c], "x6T": x6loc[c],
                           "aw5": f32("a2c5_w")[i].astype(NPBF),
                           "ab5": f32("a2c5_b")[i][:, None],
                           "aw6": f32("a2c6_w")[i].astype(NPBF),
                           "ab6": f32("a2c6_b")[i][:, None],
                           "pw5": f32("p5_w")[i].astype(NPBF),
                           "pb5": f32("p5_b")[i][:, None],
                           "pw6": f32("p6_w")[i].astype(NPBF),
                           "pb6": f32("p6_b")[i][:, None]})
        rCD = run("CD", mapsCD)
        for c in range(NC):
            x5loc[c] = np.asarray(rCD[c]["x5To"])
            x6loc[c] = np.asarray(rCD[c]["x6To"])
        x5_full = np.concatenate(
            [x5loc[c].astype(np.float32).T[:NP5] for c in range(NC)])
        x6_full = np.concatenate(
            [x6loc[c].astype(np.float32).T[:NP6] for c in range(NC)])
        # ---- E / E3
        last = (i == L - 1)
        mapsE = []
        for c, d in enumerate(prep.cores):
            u5 = x5_full[d["u5pos"]] * d["u5cs"]
            u6 = x6_full[d["u6pos"]] * d["u6cs"]
            me = {"xT": rCD[c]["xT"], "u5g": slotmajor(u5, NB * K5),
                  "u6g": slotmajor(u6, NB * K6),
                  "drel5": d["drel5"], "drel6": d["drel6"], "iotaf": prep.iotaf,
                  "w5": f32("c2a5_w")[i].astype(NPBF), "b5": f32("c2a5_b")[i][:, None],
                  "w6": f32("c2a6_w")[i].astype(NPBF), "b6": f32("c2a6_b")[i][:, None]}
            if last:
                me.update({"grel": d["grel"], "cig": d["cig"],
                           "alw": f32("atom_lin_w"), "alb": f32("atom_lin_b")[:, None],
                           "linw": f32("lin_w"), "linb": f32("lin_b")[None, :]})
            mapsE.append(me)
        rE = run("E3" if last else "E", mapsE)
        if not last:
            x_full = np.concatenate([
                np.asarray(rE[c]["xTo"]).astype(np.float32).T[:prep.nd[c]]
                for c in range(NC)])
    y = np.concatenate([rE[c]["y"][0] for c in range(NC)])[:, None]
    return y.astype(np.float32)


# revision 56
# speedup vs baseline: 1.0931x; 1.0213x over previous
"""CycleNet Trainium2 kernel: 8-core data-parallel, host-routed pipeline.

v2: host pre-gathers all random-access rows between launches (device does only
sequential DMA), feature-major layouts (no per-block transposes), bf16 matmul
paths, batched DMAs, C merged into D-launch, F merged into last E-launch.
"""
import numpy as np
import ml_dtypes
import concourse.bass as bass
import concourse.tile as tile
from concourse import bacc, mybir
from concourse.bass_utils import run_bass_kernel_spmd
from concourse.masks import make_identity

F32 = mybir.dt.float32
BF16 = mybir.dt.bfloat16
NPBF = ml_dtypes.bfloat16
P = 128
RELU = mybir.ActivationFunctionType.Relu
COPY = mybir.ActivationFunctionType.Copy
EQ = mybir.AluOpType.is_equal

# problem constants
H = 128; N = 100000; E = 250000; N5 = 20000; N6 = 30000; G = 512; L = 3
AF = 9; AV = 64; BF = 3; BV = 8; BN_EPS = 1e-5
NC = 8
GPC = G // NC            # graphs per core
NB = 104                 # node blocks per core
NDP = NB * P             # padded local nodes = 13312
KE = 4                   # edge slot tiles per node block
NET = NB * KE            # edge slot tiles per core (416)
NP5 = 12500; NP5P = 12800; D5B = 20   # local c5 positions / padded / blocks
NP6 = 22500; NP6P = 23040; D6B = 30
K5 = 2                   # u5 slot tiles per node block
K6 = 3                   # u6 slot tiles per node block

_KER_CACHE = {}


def build_G():
    """Init embeddings via multi-hot matmuls: x0T, x5T, x6T (feature-major)."""
    nc = bacc.Bacc()
    atab = nc.dram_tensor("atab", [5, P, P], BF16, kind="ExternalInput")
    mh = nc.dram_tensor("mh", [5, P, NDP], BF16, kind="ExternalInput")
    ctab = nc.dram_tensor("ctab", [16, P], BF16, kind="ExternalInput")
    mh5 = nc.dram_tensor("mh5", [16, NP5P], BF16, kind="ExternalInput")
    mh6 = nc.dram_tensor("mh6", [16, NP6P], BF16, kind="ExternalInput")
    x0T = nc.dram_tensor("x0T", [P, NDP], BF16, kind="ExternalOutput")
    x5T = nc.dram_tensor("x5T", [P, NP5P], BF16, kind="ExternalOutput")
    x6T = nc.dram_tensor("x6T", [P, NP6P], BF16, kind="ExternalOutput")
    with tile.TileContext(nc) as tc:
        with (
            tc.tile_pool(name="cons", bufs=1) as cons,
            tc.tile_pool(name="sb", bufs=3) as sb,
            tc.tile_pool(name="ps", bufs=2, space="PSUM") as ps,
        ):
            at = [cons.tile([P, P], BF16, tag=f"at{t}", name=f"at{t}") for t in range(5)]
            for t in range(5):
                nc.sync.dma_start(out=at[t][:], in_=atab[t])
            ct = cons.tile([16, P], BF16)
            nc.sync.dma_start(out=ct[:], in_=ctab[:])
            # atoms: 4-block chunks
            for g in range(NB // 4):
                mt = sb.tile([P, 5, 4 * P], BF16, tag="mt")
                nc.scalar.dma_start(
                    out=mt[:], in_=mh[:, :, g * 4 * P:(g + 1) * 4 * P].rearrange("t p h -> p t h"))
                xp = ps.tile([P, 4 * P], F32, tag="xp")
                for t in range(5):
                    nc.tensor.matmul(out=xp[:], lhsT=at[t][:], rhs=mt[:, t, :],
                                     start=(t == 0), stop=(t == 4))
                xs = sb.tile([P, 4 * P], BF16, tag="xs")
                nc.vector.tensor_copy(out=xs[:], in_=xp[:])
                nc.sync.dma_start(out=x0T[:, g * 4 * P:(g + 1) * 4 * P], in_=xs[:])
            # cycles: per 640/768-block, halves
            for k, nblk, mhk, xko in ((5, D5B, mh5, x5T), (6, D6B, mh6, x6T)):
                hp = 64 * k
                for b in range(nblk):
                    m5 = sb.tile([16, 2 * hp], BF16, tag="m5")
                    nc.gpsimd.dma_start(out=m5[:], in_=mhk[:, b * 2 * hp:(b + 1) * 2 * hp])
                    xo = sb.tile([P, 2 * hp], BF16, tag="xo5")
                    for hh in range(2):
                        cp = ps.tile([P, hp], F32, tag="cp")
                        nc.tensor.matmul(out=cp[:], lhsT=ct[:], rhs=m5[:, hh * hp:(hh + 1) * hp],
                                         start=True, stop=True)
                        nc.vector.tensor_copy(out=xo[:, hh * hp:(hh + 1) * hp], in_=cp[:])
                    nc.sync.dma_start(out=xko[:, b * 2 * hp:(b + 1) * 2 * hp], in_=xo[:])
    nc.compile()
    return nc


def build_A():
    """Edge agg (pre-gathered) + GEMM1 (x-term folded) + BN1 stats."""
    nc = bacc.Bacc()
    sg = nc.dram_tensor("sg", [P, NET // 2, 2 * P], BF16, kind="ExternalInput")
    ohb = nc.dram_tensor("ohb", [P, NET // 2, 2 * P], BF16, kind="ExternalInput")
    xT = nc.dram_tensor("xT", [P, NDP], BF16, kind="ExternalInput")
    w1 = nc.dram_tensor("w1", [P, 2 * P], BF16, kind="ExternalInput")
    w1s = nc.dram_tensor("w1s", [P, 2 * P], BF16, kind="ExternalInput")
    t1T = nc.dram_tensor("t1T", [2 * P, NDP], BF16, kind="ExternalOutput")
    bstat = nc.dram_tensor("bstat", [2, P, 2], F32, kind="ExternalOutput")
    NG = NB // 4  # 26 groups of 4 blocks
    with tile.TileContext(nc) as tc:
        with (
            tc.tile_pool(name="cons", bufs=1) as cons,
            tc.tile_pool(name="xr", bufs=1) as xr,
            tc.tile_pool(name="sb", bufs=4) as sb,
            tc.tile_pool(name="st", bufs=1) as st,
            tc.tile_pool(name="ps", bufs=2, space="PSUM") as ps,
            tc.tile_pool(name="pt", bufs=2, space="PSUM") as pt,
        ):
            w1t = cons.tile([P, 2 * P], BF16)
            nc.sync.dma_start(out=w1t[:], in_=w1[:])
            w1st = cons.tile([P, 2 * P], BF16)
            nc.sync.dma_start(out=w1st[:], in_=w1s[:])
            xt = xr.tile([P, NDP], BF16)
            for j in range(13):
                nc.scalar.dma_start(out=xt[:, j * 1024:(j + 1) * 1024],
                                    in_=xT[:, j * 1024:(j + 1) * 1024])
            stat = [st.tile([P, NG, 6], F32, tag=f"sst{h}", name=f"sst{h}") for h in range(2)]
            for g in range(NG):
                agg4 = sb.tile([P, 4 * P], BF16, tag="agg4")
                sgt = sb.tile([P, 8, 2, P], BF16, tag="sgt")
                nc.gpsimd.dma_start(
                    out=sgt[:],
                    in_=sg[:, g * 8:(g + 1) * 8, :].rearrange("p t (i h) -> p t i h", i=2))
                sl = sb.tile([P, 8, 2, P], BF16, tag="sl")
                nc.scalar.activation(out=sl[:].rearrange("p t i h -> p (t i h)"),
                                     in_=sgt[:].rearrange("p t i h -> p (t i h)"), func=RELU)
                oh = sb.tile([P, 8, 2, P], BF16, tag="oh")
                nc.sync.dma_start(
                    out=oh[:],
                    in_=ohb[:, g * 8:(g + 1) * 8, :].rearrange("p t (i h) -> p t i h", i=2))
                for bb in range(4):
                    aggT = ps.tile([P, P], F32, tag="aggT")
                    for t in range(KE):
                        tt = bb * 2 + t // 2
                        i = t % 2
                        nc.tensor.matmul(out=aggT[:], lhsT=sl[:, tt, i, :],
                                         rhs=oh[:, tt, i, :],
                                         start=(t == 0), stop=(t == KE - 1))
                    nc.scalar.activation(out=agg4[:, bb * P:(bb + 1) * P],
                                         in_=aggT[:], func=COPY)
                for half in range(2):
                    t1p = pt.tile([P, 4 * P], F32, tag=f"t1p{half}")
                    nc.tensor.matmul(out=t1p[:], lhsT=w1t[:, half * P:(half + 1) * P],
                                     rhs=agg4[:], start=True, stop=False)
                    nc.tensor.matmul(out=t1p[:], lhsT=w1st[:, half * P:(half + 1) * P],
                                     rhs=xt[:, g * 4 * P:(g + 1) * 4 * P], start=False, stop=True)
                    t1s = sb.tile([P, 4 * P], BF16, tag=f"t1s{half}")
                    nc.vector.tensor_copy(out=t1s[:], in_=t1p[:])
                    nc.vector.bn_stats(out=stat[half][:, g, :], in_=t1s[:])
                    nc.sync.dma_start(out=t1T[half * P:(half + 1) * P, g * 4 * P:(g + 1) * 4 * P],
                                      in_=t1s[:])
            for half in range(2):
                mv = sb.tile([P, 2], F32, tag="mv")
                nc.vector.bn_aggr(out=mv[:], in_=stat[half][:])
                nc.sync.dma_start(out=bstat[half], in_=mv[:])
    nc.compile()
    return nc


def build_B():
    """t2 = relu(t1*a1+b1); hT = w2-chain; BN2 stats."""
    nc = bacc.Bacc()
    t1T = nc.dram_tensor("t1T", [2 * P, NDP], BF16, kind="ExternalInput")
    ab1 = nc.dram_tensor("ab1", [2, 2, P, 1], F32, kind="ExternalInput")
    gw2 = nc.dram_tensor("gw2", [2 * P, P], BF16, kind="ExternalInput")
    hT = nc.dram_tensor("hT", [P, NDP], BF16, kind="ExternalOutput")
    bstat = nc.dram_tensor("bstat", [P, 2], F32, kind="ExternalOutput")
    NG = NB // 4
    with tile.TileContext(nc) as tc:
        with (
            tc.tile_pool(name="cons", bufs=1) as cons,
            tc.tile_pool(name="sb", bufs=4) as sb,
            tc.tile_pool(name="st", bufs=1) as st,
            tc.tile_pool(name="ps", bufs=2, space="PSUM") as ps,
        ):
            w2 = [cons.tile([P, P], BF16, tag=f"w2{h}", name=f"w2{h}") for h in range(2)]
            a1 = [cons.tile([P, 1], F32, tag=f"a{h}", name=f"a1_{h}") for h in range(2)]
            b1 = [cons.tile([P, 1], F32, tag=f"b{h}", name=f"b1_{h}") for h in range(2)]
            for h in range(2):
                nc.sync.dma_start(out=w2[h][:], in_=gw2[h * P:(h + 1) * P, :])
                nc.sync.dma_start(out=a1[h][:], in_=ab1[h, 0])
                nc.sync.dma_start(out=b1[h][:], in_=ab1[h, 1])
            stat = st.tile([P, NG, 6], F32)
            for g in range(NG):
                hp = ps.tile([P, 4 * P], F32, tag="hp")
                for half in range(2):
                    t1s = sb.tile([P, 4 * P], BF16, tag=f"t1s{half}")
                    (nc.scalar if half == 0 else nc.gpsimd).dma_start(
                        out=t1s[:], in_=t1T[half * P:(half + 1) * P,
                                            g * 4 * P:(g + 1) * 4 * P])
                    t2s = sb.tile([P, 4 * P], BF16, tag=f"t2s{half}")
                    nc.scalar.activation(out=t2s[:], in_=t1s[:], func=RELU,
                                         bias=b1[half][:], scale=a1[half][:])
                    nc.tensor.matmul(out=hp[:], lhsT=w2[half][:], rhs=t2s[:],
                                     start=(half == 0), stop=(half == 1))
                hs = sb.tile([P, 4 * P], BF16, tag="hs")
                nc.vector.tensor_copy(out=hs[:], in_=hp[:])
                nc.vector.bn_stats(out=stat[:, g, :], in_=hs[:])
                nc.sync.dma_start(out=hT[:, g * 4 * P:(g + 1) * 4 * P], in_=hs[:])
            mv = sb.tile([P, 2], F32, tag="mv")
            nc.vector.bn_aggr(out=mv[:], in_=stat[:])
            nc.sync.dma_start(out=bstat[:], in_=mv[:])
    nc.compile()
    return nc


def build_CD():
    """BN2-apply (xT out) + a2c mixes + cyclic path blocks for c5 and c6."""
    nc = bacc.Bacc()
    hT = nc.dram_tensor("hT", [P, NDP], BF16, kind="ExternalInput")
    ab2 = nc.dram_tensor("ab2", [2, P, 1], F32, kind="ExternalInput")
    z5g = nc.dram_tensor("z5g", [P, NP5P], BF16, kind="ExternalInput")
    z6g = nc.dram_tensor("z6g", [P, NP6P], BF16, kind="ExternalInput")
    x5T = nc.dram_tensor("x5T", [P, NP5P], BF16, kind="ExternalInput")
    x6T = nc.dram_tensor("x6T", [P, NP6P], BF16, kind="ExternalInput")
    aw5 = nc.dram_tensor("aw5", [P, P], BF16, kind="ExternalInput")
    ab5 = nc.dram_tensor("ab5", [P, 1], F32, kind="ExternalInput")
    aw6 = nc.dram_tensor("aw6", [P, P], BF16, kind="ExternalInput")
    ab6 = nc.dram_tensor("ab6", [P, 1], F32, kind="ExternalInput")
    pw5 = nc.dram_tensor("pw5", [3, P, P], BF16, kind="ExternalInput")
    pb5 = nc.dram_tensor("pb5", [P, 1], F32, kind="ExternalInput")
    pw6 = nc.dram_tensor("pw6", [3, P, P], BF16, kind="ExternalInput")
    pb6 = nc.dram_tensor("pb6", [P, 1], F32, kind="ExternalInput")
    xT = nc.dram_tensor("xT", [P, NDP], BF16, kind="ExternalOutput")
    x5To = nc.dram_tensor("x5To", [P, NP5P], BF16, kind="ExternalOutput")
    x6To = nc.dram_tensor("x6To", [P, NP6P], BF16, kind="ExternalOutput")
    with tile.TileContext(nc) as tc:
        with (
            tc.tile_pool(name="cons", bufs=1) as cons,
            tc.tile_pool(name="sb", bufs=4) as sb,
            tc.tile_pool(name="ps", bufs=3, space="PSUM") as ps,
        ):
            a2 = cons.tile([P, 1], F32)
            nc.sync.dma_start(out=a2[:], in_=ab2[0])
            b2 = cons.tile([P, 1], F32)
            nc.sync.dma_start(out=b2[:], in_=ab2[1])
            awt = {5: cons.tile([P, P], BF16, name="aw5t"), 6: cons.tile([P, P], BF16, name="aw6t")}
            abt = {5: cons.tile([P, 1], F32, name="ab5t"), 6: cons.tile([P, 1], F32, name="ab6t")}
            pwt = {5: [cons.tile([P, P], BF16, name=f"pw5{s}") for s in range(3)],
                   6: [cons.tile([P, P], BF16, name=f"pw6{s}") for s in range(3)]}
            pbt = {5: cons.tile([P, 1], F32, name="pb5t"), 6: cons.tile([P, 1], F32, name="pb6t")}
            for k, aws, abs_, pws, pbs in ((5, aw5, ab5, pw5, pb5), (6, aw6, ab6, pw6, pb6)):
                nc.sync.dma_start(out=awt[k][:], in_=aws[:])
                nc.sync.dma_start(out=abt[k][:], in_=abs_[:])
                for s in range(3):
                    nc.sync.dma_start(out=pwt[k][s][:], in_=pws[s])
                nc.sync.dma_start(out=pbt[k][:], in_=pbs[:])
            # C part: xT = relu(a2*hT + b2)
            for j in range(13):
                hs = sb.tile([P, 1024], BF16, tag="hs")
                nc.scalar.dma_start(out=hs[:], in_=hT[:, j * 1024:(j + 1) * 1024])
                xs = sb.tile([P, 1024], BF16, tag="xs")
                nc.scalar.activation(out=xs[:], in_=hs[:], func=RELU,
                                     bias=b2[:], scale=a2[:])
                nc.sync.dma_start(out=xT[:, j * 1024:(j + 1) * 1024], in_=xs[:])
            # D part per k: halo-layout conv, bf16 state
            for k, nblk, zg, xk, xko in ((5, D5B, z5g, x5T, x5To), (6, D6B, z6g, x6T, x6To)):
                BPOS = P * k
                HP = 64 * k
                for blk in range(nblk):
                    if blk % 2 == 0:
                        zt2 = sb.tile([P, 2 * BPOS], BF16, tag="zt")
                        nc.gpsimd.dma_start(out=zt2[:], in_=zg[:, blk * BPOS:(blk + 2) * BPOS])
                        x5b2 = sb.tile([P, 2 * BPOS], BF16, tag="x5b")
                        nc.gpsimd.dma_start(out=x5b2[:], in_=xk[:, blk * BPOS:(blk + 2) * BPOS])
                        xo2 = sb.tile([P, 2 * BPOS], BF16, tag="xo")
                    off = (blk % 2) * BPOS
                    zt = zt2[:, off:off + BPOS]
                    x5b = x5b2[:, off:off + BPOS]
                    xo = xo2[:, off:off + BPOS]
                    for hh in range(2):
                        zb = sb.tile([P, HP], BF16, tag="zb")
                        nc.scalar.activation(out=zb[:], in_=zt[:, hh * HP:(hh + 1) * HP],
                                             func=RELU, bias=b2[:], scale=a2[:])
                        rp = ps.tile([P, HP], F32, tag="rp")
                        nc.tensor.matmul(out=rp[:], lhsT=awt[k][:], rhs=zb[:],
                                         start=True, stop=True)
                        rs = sb.tile([P, HP], F32, tag="rs")
                        nc.scalar.activation(out=rs[:], in_=rp[:], func=RELU, bias=abt[k][:])
                        xv3 = x5b[:, hh * HP:(hh + 1) * HP].rearrange("h (c j) -> h c j", j=k)
                        xch = sb.tile([P, 64, k + 2], BF16, tag="xch")
                        nc.vector.tensor_add(out=xch[:, :, 1:k + 1], in0=xv3,
                                             in1=rs[:].rearrange("h (c j) -> h c j", j=k))
                        nc.vector.tensor_copy(out=xch[:, :, 0:1], in_=xch[:, :, k:k + 1])
                        nc.vector.tensor_copy(out=xch[:, :, k + 1:k + 2], in_=xch[:, :, 1:2])
                        cvp = ps.tile([P, HP], F32, tag="cvp")
                        for s in range(3):
                            nc.tensor.matmul(out=cvp[:], lhsT=pwt[k][s][:],
                                             rhs=xch[:, :, s:s + k], start=(s == 0), stop=(s == 2))
                        cvr = sb.tile([P, HP], F32, tag="cvr")
                        nc.scalar.activation(out=cvr[:], in_=cvp[:], func=RELU, bias=pbt[k][:])
                        nc.vector.tensor_add(
                            out=xo[:, hh * HP:(hh + 1) * HP].rearrange("h (c j) -> h c j", j=k),
                            in0=xch[:, :, 1:k + 1], in1=cvr[:].rearrange("h (c j) -> h c j", j=k))
                    if blk % 2 == 1:
                        nc.sync.dma_start(out=xko[:, (blk - 1) * BPOS:(blk + 1) * BPOS],
                                          in_=xo2[:])
    nc.compile()
    return nc


def build_E(readout):
    """c2a: seg-mean (pre-gathered, pre-scaled) + linear + relu + residual.
    readout=True: fold the graph readout (F) in instead of storing xT'."""
    nc = bacc.Bacc()
    xT = nc.dram_tensor("xT", [P, NDP], BF16, kind="ExternalInput")
    u5g = nc.dram_tensor("u5g", [P, NB * K5 // 2, 2 * P], BF16, kind="ExternalInput")
    u6g = nc.dram_tensor("u6g", [P, NB * K6 // 2, 2 * P], BF16, kind="ExternalInput")
    drel5 = nc.dram_tensor("drel5", [P, NB * K5], F32, kind="ExternalInput")
    drel6 = nc.dram_tensor("drel6", [P, NB * K6], F32, kind="ExternalInput")
    iotaf = nc.dram_tensor("iotaf", [P, 8 * P], F32, kind="ExternalInput")
    w5 = nc.dram_tensor("w5", [P, P], BF16, kind="ExternalInput")
    b5 = nc.dram_tensor("b5", [P, 1], F32, kind="ExternalInput")
    w6 = nc.dram_tensor("w6", [P, P], BF16, kind="ExternalInput")
    b6 = nc.dram_tensor("b6", [P, 1], F32, kind="ExternalInput")
    if readout:
        grel = nc.dram_tensor("grel", [P, NB], F32, kind="ExternalInput")
        cig = nc.dram_tensor("cig", [P, GPC], F32, kind="ExternalInput")
        alw = nc.dram_tensor("alw", [P, P], F32, kind="ExternalInput")
        alb = nc.dram_tensor("alb", [P, 1], F32, kind="ExternalInput")
        linw = nc.dram_tensor("linw", [P, 1], F32, kind="ExternalInput")
        linb = nc.dram_tensor("linb", [1, 1], F32, kind="ExternalInput")
        y = nc.dram_tensor("y", [1, GPC], F32, kind="ExternalOutput")
    else:
        xTo = nc.dram_tensor("xTo", [P, NDP], BF16, kind="ExternalOutput")
    with tile.TileContext(nc) as tc:
        with (
            tc.tile_pool(name="cons", bufs=1) as cons,
            tc.tile_pool(name="sb", bufs=4) as sb,
            tc.tile_pool(name="psa", bufs=(1 if readout else 2), space="PSUM") as psa,
            tc.tile_pool(name="psb", bufs=(3 if readout else 2), space="PSUM") as psb,
            tc.tile_pool(name="psg", bufs=1, space="PSUM") as psg,
        ):
            iot8 = cons.tile([P, 8, P], F32)
            nc.sync.dma_start(out=iot8[:], in_=iotaf[:].rearrange("p (k h) -> p k h", k=8))
            dr5 = cons.tile([P, NB * K5], F32)
            nc.sync.dma_start(out=dr5[:], in_=drel5[:])
            dr6 = cons.tile([P, NB * K6], F32)
            nc.sync.dma_start(out=dr6[:], in_=drel6[:])
            wt = {5: cons.tile([P, P], BF16, name="w5t"), 6: cons.tile([P, P], BF16, name="w6t")}
            bt = {5: cons.tile([P, 1], F32, name="b5t"), 6: cons.tile([P, 1], F32, name="b6t")}
            nc.sync.dma_start(out=wt[5][:], in_=w5[:])
            nc.sync.dma_start(out=bt[5][:], in_=b5[:])
            nc.sync.dma_start(out=wt[6][:], in_=w6[:])
            nc.sync.dma_start(out=bt[6][:], in_=b6[:])
            if readout:
                ident = cons.tile([P, P], F32)
                make_identity(nc, ident[:])
                gr = cons.tile([P, NB], F32)
                nc.sync.dma_start(out=gr[:], in_=grel[:])
                cigt = cons.tile([P, GPC], F32)
                nc.sync.dma_start(out=cigt[:], in_=cig[:])
                alwt = cons.tile([P, P], F32)
                nc.sync.dma_start(out=alwt[:], in_=alw[:])
                albt = cons.tile([P, 1], F32)
                nc.sync.dma_start(out=albt[:], in_=alb[:])
                linwt = cons.tile([P, 1], F32)
                nc.sync.dma_start(out=linwt[:], in_=linw[:])
                linbt = cons.tile([1, 1], F32)
                nc.sync.dma_start(out=linbt[:], in_=linb[:])
                xgT = psg.tile([P, GPC], F32)
            for b in range(NB):
                if b % 4 == 0:
                    u5t = sb.tile([P, 4, 2, P], BF16, tag="u5t")
                    nc.scalar.dma_start(
                        out=u5t[:],
                        in_=u5g[:, b:b + 4, :].rearrange("p t (i h) -> p t i h", i=2))
                    xb = sb.tile([P, 4 * P], BF16, tag="xb")
                    nc.gpsimd.dma_start(out=xb[:], in_=xT[:, b * P:(b + 4) * P])
                    if not readout:
                        xno = sb.tile([P, 4 * P], BF16, tag="xno")
                    u6t = sb.tile([P, 6, 2, P], BF16, tag="u6t")
                    nc.gpsimd.dma_start(
                        out=u6t[:],
                        in_=u6g[:, b * 3 // 2:b * 3 // 2 + 6, :].rearrange(
                            "p t (i h) -> p t i h", i=2))
                rr = {}
                for k, K, ut, drk in ((5, K5, u5t, dr5), (6, K6, u6t, dr6)):
                    uT = psa.tile([P, P], F32, tag=f"uT{k}")
                    oh = sb.tile([P, K, P], BF16, tag=f"oh{k}")
                    nc.vector.tensor_tensor(
                        out=oh[:], in0=drk[:, b * K:(b + 1) * K].to_broadcast([P, K, P]),
                        in1=iot8[:, 0:K, :], op=EQ)
                    for t in range(K):
                        lt = (b % 4) * K + t
                        nc.tensor.matmul(out=uT[:], lhsT=ut[:, lt // 2, lt % 2, :],
                                         rhs=oh[:, t, :],
                                         start=(t == 0), stop=(t == K - 1))
                    us = sb.tile([P, P], BF16, tag=f"us{k}")
                    if k == 5:
                        nc.vector.tensor_copy(out=us[:], in_=uT[:])
                    else:
                        nc.scalar.activation(out=us[:], in_=uT[:], func=COPY)
                    rp = psb.tile([P, P], F32, tag="rp")
                    nc.tensor.matmul(out=rp[:], lhsT=wt[k][:], rhs=us[:], start=True, stop=True)
                    rs = sb.tile([P, P], BF16, tag=f"rs{k}")
                    nc.scalar.activation(out=rs[:], in_=rp[:], func=RELU, bias=bt[k][:])
                    rr[k] = rs
                xn = sb.tile([P, P], BF16, tag="xn")
                nc.gpsimd.tensor_add(out=xn[:], in0=xb[:, (b % 4) * P:(b % 4 + 1) * P],
                                     in1=rr[5][:])
                if readout:
                    nc.vector.tensor_add(out=xn[:], in0=xn[:], in1=rr[6][:])
                    xf = sb.tile([P, P], F32, tag="xf")
                    nc.scalar.activation(out=xf[:], in_=xn[:], func=COPY)
                    tp = psb.tile([P, P], F32, tag="rp")
                    nc.tensor.transpose(out=tp[:], in_=xf[:], identity=ident[:])
                    xfT = sb.tile([P, P], F32, tag="xfT")
                    nc.vector.tensor_copy(out=xfT[:], in_=tp[:])
                    ohg = sb.tile([P, GPC], F32, tag="ohg")
                    nc.vector.tensor_tensor(out=ohg[:], in0=gr[:, b:b + 1].to_broadcast([P, GPC]),
                                            in1=iot8[:, 0, :GPC], op=EQ)
                    nc.tensor.matmul(out=xgT[:], lhsT=xfT[:], rhs=ohg[:],
                                     start=(b == 0), stop=(b == NB - 1))
                else:
                    nc.vector.tensor_add(out=xno[:, (b % 4) * P:(b % 4 + 1) * P],
                                         in0=xn[:], in1=rr[6][:])
                    if b % 4 == 3:
                        nc.sync.dma_start(out=xTo[:, (b - 3) * P:(b + 1) * P], in_=xno[:])
            if readout:
                xg = sb.tile([P, GPC], F32, tag="xg")
                nc.vector.tensor_mul(out=xg[:], in0=xgT[:], in1=cigt[:])
                ap = psg.tile([P, GPC], F32, tag="ap")
                nc.tensor.matmul(out=ap[:], lhsT=alwt[:], rhs=xg[:], start=True, stop=True)
                av = sb.tile([P, GPC], F32, tag="av")
                nc.scalar.activation(out=av[:], in_=ap[:], func=RELU, bias=albt[:])
                yp = psg.tile([1, GPC], F32, tag="yp")
                nc.tensor.matmul(out=yp[:], lhsT=linwt[:], rhs=av[:], start=True, stop=True)
                ys = sb.tile([1, GPC], F32, tag="ys")
                nc.vector.tensor_scalar_add(out=ys[:], in0=yp[:], scalar1=linbt[:])
                nc.sync.dma_start(out=y[:], in_=ys[:])
    nc.compile()
    return nc


def get_kernels():
    if "G" not in _KER_CACHE:
        _KER_CACHE.update(G=build_G(), A=build_A(), B=build_B(), CD=build_CD(),
                          E=build_E(False), E3=build_E(True))
    return _KER_CACHE


# ---------------------------------------------------------------- host glue

def slotmajor(vals, ntiles):
    """[ntiles*128, H] -> [128, ntiles//2, 2*H] bf16 (pair-interleaved)."""
    return np.ascontiguousarray(
        vals.reshape(ntiles // 2, 2, P, H).transpose(2, 0, 1, 3).reshape(
            P, ntiles // 2, 2 * H)).astype(NPBF)


class Prep:
    """Per-core layer-invariant index prep."""

    def __init__(self, x_atom, edge_index, edge_attr, batch, xc5, xc6, r5, r6):
        self.iotaf = np.tile(np.arange(P, dtype=np.float32)[None, :], (P, 8))
        core_of_node = (batch // GPC).astype(np.int64)
        self.node_lo = np.searchsorted(batch, np.arange(NC) * GPC)
        self.node_hi = np.searchsorted(batch, np.arange(NC) * GPC + GPC)
        self.nd = self.node_hi - self.node_lo
        assert self.nd.max() <= NDP
        src, dst = edge_index[0], edge_index[1]
        combo = (edge_attr[:, 0] * (BV * BV) + edge_attr[:, 1] * BV + edge_attr[:, 2])
        self.cores = []
        for c in range(NC):
            d = {}
            lo, hi, nd = self.node_lo[c], self.node_hi[c], self.nd[c]
            # ---- edge slots grouped by dst block
            em = np.where(core_of_node[dst] == c)[0]
            eblk = (dst[em] - lo) // P
            order = np.argsort(eblk, kind="stable")
            em = em[order]; eblk = eblk[order]
            cnt = np.bincount(eblk, minlength=NB)
            assert cnt.max() <= KE * P, f"edge block overflow {cnt.max()}"
            nslot = NET * P
            slot_src = np.zeros(nslot, dtype=np.int64)
            slot_ea = np.full(nslot, 512, dtype=np.int64)
            slot_dr = np.full(nslot, 255.0, dtype=np.float32)
            starts = np.concatenate([[0], np.cumsum(cnt)])
            for b in range(NB):
                sl = b * KE * P
                e = em[starts[b]:starts[b + 1]]
                slot_src[sl:sl + len(e)] = src[e]
                slot_ea[sl:sl + len(e)] = combo[e]
                slot_dr[sl:sl + len(e)] = (dst[e] - lo - b * P).astype(np.float32)
            d["slot_src"] = slot_src
            d["slot_ea"] = slot_ea
            dstrel = np.ascontiguousarray(slot_dr.reshape(NET, P).T)
            d["ohb"] = (dstrel[:, :, None]
                        == np.arange(P, dtype=np.float32)[None, None, :]).astype(
                            NPBF).reshape(P, NET // 2, 2 * P)
            # ---- z rows (a2c sources): global node ids per local cycle position
            for kk, npos, nposp, rows_all in ((5, NP5, NP5P, r5), (6, NP6, NP6P, r6)):
                rp = np.zeros(nposp, dtype=np.int64)
                rp[:npos] = rows_all[c * npos:(c + 1) * npos]
                d[f"z{kk}rows"] = rp
                d[f"z{kk}mask"] = npos
            # ---- u slots (c2a): positions targeting this core's nodes
            cnt5 = np.bincount(r5, minlength=N).astype(np.float32)
            cnt6 = np.bincount(r6, minlength=N).astype(np.float32)
            for kk, rows_all, K, cnt_node in ((5, r5, K5, cnt5), (6, r6, K6, cnt6)):
                pm = np.where(core_of_node[rows_all] == c)[0]
                tblk = (rows_all[pm] - lo) // P
                order = np.argsort(tblk, kind="stable")
                pm = pm[order]; tblk = tblk[order]
                cntb = np.bincount(tblk, minlength=NB)
                assert cntb.max() <= K * P, f"u{kk} block overflow {cntb.max()}"
                nslot = NB * K * P
                slot_pos = np.zeros(nslot, dtype=np.int64)
                slot_dr = np.full(nslot, 255.0, dtype=np.float32)
                slot_cs = np.zeros(nslot, dtype=np.float32)
                cinv = 1.0 / np.maximum(cnt_node, 1.0)
                st = np.concatenate([[0], np.cumsum(cntb)])
                for b in range(NB):
                    sl = b * K * P
                    pp = pm[st[b]:st[b + 1]]
                    slot_pos[sl:sl + len(pp)] = pp
                    slot_dr[sl:sl + len(pp)] = (rows_all[pp] - lo - b * P).astype(np.float32)
                    slot_cs[sl:sl + len(pp)] = cinv[rows_all[pp]]
                d[f"u{kk}pos"] = slot_pos
                d[f"u{kk}cs"] = slot_cs[:, None]
                d[f"drel{kk}"] = np.ascontiguousarray(slot_dr.reshape(NB * K, P).T)
            # ---- init multi-hots
            mh = np.zeros((640, NDP), dtype=np.float32)
            colr = np.arange(nd)
            for f in range(AF):
                mh[f * AV + x_atom[lo:hi, f], colr] = 1.0
            d["mh"] = np.ascontiguousarray(mh.reshape(5, P, NDP)).astype(NPBF)
            mh5 = np.zeros((16, NP5P), dtype=np.float32)
            mh5[xc5[c * NP5:(c + 1) * NP5], np.arange(NP5)] = 1.0
            d["mh5"] = mh5.astype(NPBF)
            mh6 = np.zeros((16, NP6P), dtype=np.float32)
            mh6[4 + xc6[c * NP6:(c + 1) * NP6], np.arange(NP6)] = 1.0
            d["mh6"] = mh6.astype(NPBF)
            # ---- readout
            grel = np.full((NB * P,), 255.0, dtype=np.float32)
            grel[:nd] = (batch[lo:hi] - c * GPC).astype(np.float32)
            d["grel"] = np.ascontiguousarray(grel.reshape(NB, P).T)
            gsz = np.bincount(batch, minlength=G).astype(np.float32)[c * GPC:(c + 1) * GPC]
            d["cig"] = np.tile(1.0 / np.maximum(gsz, 1.0)[None, :], (P, 1))
            self.cores.append(d)


def _run(nc, in_maps, trace=False):
    return run_bass_kernel_spmd(nc, in_maps, core_ids=list(range(NC)), trace=trace)


_EXEC_NS = []  # exec_time_ns per launch when tracing


def kernel(**inputs):
    inp = {k: np.asarray(v) for k, v in inputs.items()}
    x_atom = inp["x_atom"].astype(np.int64)
    edge_index = inp["edge_index"].astype(np.int64)
    edge_attr = inp["edge_attr"].astype(np.int64)
    batch = inp["batch"].astype(np.int64)
    xc5 = inp["xc5"].astype(np.int64); xc6 = inp["xc6"].astype(np.int64)
    r5 = inp["a2c5_row"].astype(np.int64); r6 = inp["a2c6_row"].astype(np.int64)
    f32 = lambda k: inp[k].astype(np.float32)
    atom_emb = f32("atom_emb"); bond_emb = f32("bond_emb")
    cyc5 = f32("cyc_emb5"); cyc6 = f32("cyc_emb6"); eps = f32("gine_eps")
    gw1 = f32("gw1"); gbn_g = f32("gbn_g"); gbn_b = f32("gbn_b")
    gw2 = f32("gw2"); bn_g = f32("bn_g"); bn_b = f32("bn_b")
    trace = bool(int(__import__("os").environ.get("CYC_TRACE", "0")))

    prep = Prep(x_atom, edge_index, edge_attr, batch, xc5, xc6, r5, r6)
    ks = get_kernels()
    _EXEC_NS.clear()

    def run(name, maps):
        res = _run(ks[name], maps, trace=trace)
        if trace and res.exec_time_ns is not None:
            _EXEC_NS.append((name, res.exec_time_ns))
        return res.results

    # ---- init embeddings
    atab = np.zeros((640, H), np.float32)
    atab[:AF * AV] = atom_emb.reshape(AF * AV, H)
    atab = np.ascontiguousarray(atab.reshape(5, P, H)).astype(NPBF)
    ctab = np.zeros((16, H), np.float32)
    ctab[0:4] = cyc5; ctab[4:8] = cyc6
    ctab = ctab.astype(NPBF)
    rG = run("G", [{"atab": atab, "ctab": ctab, "mh": d["mh"],
                    "mh5": d["mh5"], "mh6": d["mh6"]} for d in prep.cores])
    x_full = np.concatenate([
        np.asarray(rG[c]["x0T"]).astype(np.float32).T[:prep.nd[c]] for c in range(NC)])
    x5loc = [np.asarray(rG[c]["x5T"]) for c in range(NC)]
    x6loc = [np.asarray(rG[c]["x6T"]) for c in range(NC)]

    def xT_of(xf):
        """x_full [N,H] -> per-core zero-padded feature-major bf16 [P, NDP]."""
        outs = []
        for c in range(NC):
            m = np.zeros((NDP, H), np.float32)
            m[:prep.nd[c]] = xf[prep.node_lo[c]:prep.node_hi[c]]
            outs.append(np.ascontiguousarray(m.T).astype(NPBF))
        return outs

    for i in range(L):
        xTs = xT_of(x_full)
        be = bond_emb[i]
        combos = np.arange(BV ** 3)
        etab = (be[0][combos // (BV * BV)] + be[1][(combos // BV) % BV] + be[2][combos % BV])
        etab = np.concatenate([etab, np.zeros((1, H), np.float32)])
        w1 = gw1[i].astype(NPBF)
        w1s = (gw1[i] * (1.0 + eps[i])).astype(NPBF)
        # ---- A
        mapsA = []
        for c, d in enumerate(prep.cores):
            vals = x_full[d["slot_src"]] + etab[d["slot_ea"]]
            mapsA.append({"sg": slotmajor(vals, NET), "ohb": d["ohb"],
                          "xT": xTs[c], "w1": w1, "w1s": w1s})
        rA = run("A", mapsA)
        m = np.stack([np.concatenate([rA[c]["bstat"][0, :, 0], rA[c]["bstat"][1, :, 0]])
                      for c in range(NC)]).astype(np.float64)
        v = np.stack([np.concatenate([rA[c]["bstat"][0, :, 1], rA[c]["bstat"][1, :, 1]])
                      for c in range(NC)]).astype(np.float64)
        tot = m.sum(0) * NDP
        tot2 = (v + m ** 2).sum(0) * NDP
        m1 = tot / N
        v1 = tot2 / N - m1 ** 2
        a1 = (gbn_g[i] / np.sqrt(v1 + BN_EPS)).astype(np.float32)
        b1 = (gbn_b[i] - a1 * m1).astype(np.float32)
        ab1 = np.stack([np.stack([a1[h * P:(h + 1) * P, None], b1[h * P:(h + 1) * P, None]])
                        for h in range(2)])
        # ---- B
        rB = run("B", [{"t1T": rA[c]["t1T"], "ab1": ab1, "gw2": gw2[i].astype(NPBF)}
                       for c in range(NC)])
        m2 = np.stack([rB[c]["bstat"][:, 0] for c in range(NC)]).astype(np.float64)
        v2 = np.stack([rB[c]["bstat"][:, 1] for c in range(NC)]).astype(np.float64)
        hpad = (np.maximum(b1, 0.0).astype(np.float64) @ gw2[i].astype(np.float64))
        npad = NC * NDP - N
        tot = m2.sum(0) * NDP - npad * hpad
        tot2 = (v2 + m2 ** 2).sum(0) * NDP - npad * hpad ** 2
        m2g = tot / N
        v2g = tot2 / N - m2g ** 2
        a2 = (bn_g[i] / np.sqrt(v2g + BN_EPS)).astype(np.float32)
        b2 = (bn_b[i] - a2 * m2g).astype(np.float32)
        ab2 = np.stack([a2[:, None], b2[:, None]])
        # ---- CD
        h_full = np.concatenate([
            np.asarray(rB[c]["hT"]).astype(np.float32).T[:prep.nd[c]] for c in range(NC)])
        mapsCD = []
        for c, d in enumerate(prep.cores):
            z5 = np.ascontiguousarray(h_full[d["z5rows"]].T).astype(NPBF)
            z6 = np.ascontiguousarray(h_full[d["z6rows"]].T).astype(NPBF)
            mapsCD.append({"hT": rB[c]["hT"], "ab2": ab2, "z5g": z5, "z6g": z6,
                           "x5T": x5loc[c], "x6T": x6loc[c],
                           "aw5": f32("a2c5_w")[i].astype(NPBF),
                           "ab5": f32("a2c5_b")[i][:, None],
                           "aw6": f32("a2c6_w")[i].astype(NPBF),
                           "ab6": f32("a2c6_b")[i][:, None],
                           "pw5": f32("p5_w")[i].astype(NPBF),
                           "pb5": f32("p5_b")[i][:, None],
                           "pw6": f32("p6_w")[i].astype(NPBF),
                           "pb6": f32("p6_b")[i][:, None]})
        rCD = run("CD", mapsCD)
        for c in range(NC):
            x5loc[c] = np.asarray(rCD[c]["x5To"])
            x6loc[c] = np.asarray(rCD[c]["x6To"])
        x5_full = np.concatenate(
            [x5loc[c].astype(np.float32).T[:NP5] for c in range(NC)])
        x6_full = np.concatenate(
            [x6loc[c].astype(np.float32).T[:NP6] for c in range(NC)])
        # ---- E / E3
        last = (i == L - 1)
        mapsE = []
        for c, d in enumerate(prep.cores):
            u5 = x5_full[d["u5pos"]] * d["u5cs"]
            u6 = x6_full[d["u6pos"]] * d["u6cs"]
            me = {"xT": rCD[c]["xT"], "u5g": slotmajor(u5, NB * K5),
                  "u6g": slotmajor(u6, NB * K6),
                  "drel5": d["drel5"], "drel6": d["drel6"], "iotaf": prep.iotaf,
                  "w5": f32("c2a5_w")[i].astype(NPBF), "b5": f32("c2a5_b")[i][:, None],
                  "w6": f32("c2a6_w")[i].astype(NPBF), "b6": f32("c2a6_b")[i][:, None]}
            if last:
                me.update({"grel": d["grel"], "cig": d["cig"],
                           "alw": f32("atom_lin_w"), "alb": f32("atom_lin_b")[:, None],
                           "linw": f32("lin_w"), "linb": f32("lin_b")[None, :]})
            mapsE.append(me)
        rE = run("E3" if last else "E", mapsE)
        if not last:
            x_full = np.concatenate([
                np.asarray(rE[c]["xTo"]).astype(np.float32).T[:prep.nd[c]]
                for c in range(NC)])
    y = np.concatenate([rE[c]["y"][0] for c in range(NC)])[:, None]
    return y.astype(np.float32)
